# revision 16
# baseline (speedup 1.0000x reference)
"""Trainium2 Bass kernel for nn_DevLayer_12627203850761 (moe_routing).

Strategy:
  - Batch-parallel across 8 NeuronCores: core c processes batch element c
    of both streams (emb + dis). No collectives needed (routing top-2 and
    per-block weight gather/folding done host-side; `delayed` is a
    per-batch mean so it is core-local).
  - On device, activations live feature-major ([D partitions, T free]) in
    bf16; all matmuls use the weights as the stationary lhsT operand.
  - LayerNorm mean/var are computed with ones-matmuls on the PE (free on
    the bottleneck-adjacent engines), rstd via a bit-hack+Newton rsqrt on
    the vector engine (ACT Rsqrt is banned), broadcast back over
    partitions with a K=1 ones-matmul.
  - LN gamma/beta, biases, torsion factors and the 0.5/0.3 residual
    scales are folded into the weights / per-feature bias vectors on the
    host, so the device only does: stats, center, scale, matmul,
    ACT(tanh/gelu) with per-partition bias, and one fused
    scalar_tensor_tensor per residual add.
  - Layout changes (token-major f32 DRAM <-> feature-major bf16 SBUF) are
    done purely with DMA: SWDGE cast-DMA (f32<->bf16) + HWDGE xbar
    transpose (2-byte dtype).
"""

import sys
import numpy as np

if '/opt/trn_rl_repo' not in sys.path:
    sys.path.insert(0, '/opt/trn_rl_repo')

B, S, D, NB = 8, 8192, 512, 16
P = 128
KB = D // P            # 4 feature blocks
TC = 512               # token chunk (PSUM free dim)
EPS = 1e-5
NEWTON_ITERS = 2
N_CORES = 8
GELU_FUNC_NAME = "Gelu"   # CoreSim has no Gelu; sim tests swap in "Tanh"

_MODULE_CACHE = {}


# ----------------------------------------------------------------------------
# Host-side routing + weight folding
# ----------------------------------------------------------------------------

def _top2(scores_row):
    # jax.lax.top_k: descending values, ties -> lower index first
    idx = np.lexsort((np.arange(scores_row.shape[0]), -scores_row))
    return int(idx[0]), int(idx[1])


def _prep_host(inputs):
    """Compute routing and folded per-core device inputs."""
    f32 = np.float32
    emb_input = np.asarray(inputs["emb_input"], f32)
    dis_input = np.asarray(inputs["dis_input"], f32)
    torsion = np.asarray(inputs["torsion"], f32)
    dis_on = bool(int(inputs["dis_unlocked"]))

    # ---- routing (sigmoid is monotonic -> top_k on logits)
    m0 = emb_input[0].mean(axis=0, dtype=f32)                       # [D]
    es = m0 @ np.asarray(inputs["emb_sel_W"], f32) + np.asarray(inputs["emb_sel_b"], f32)
    etop = _top2(es)

    # ---- emb folded weights (shared across cores)
    w_e1 = np.empty((2, D, D), f32)
    b_e1 = np.empty((2, D), f32)
    w_e2_base = np.empty((2, D, D), f32)
    b_e2_base = np.empty((2, D), f32)
    for i, idx in enumerate(etop):
        g = np.asarray(inputs["emb_ln_g"], f32)[idx]
        b = np.asarray(inputs["emb_ln_b"], f32)[idx]
        w1 = np.asarray(inputs["emb_w1"], f32)[idx]
        w_e1[i] = g[:, None] * w1
        b_e1[i] = b @ w1 + np.asarray(inputs["emb_b1"], f32)[idx]
        w_e2_base[i] = np.asarray(inputs["emb_w2"], f32)[idx]
        b_e2_base[i] = np.asarray(inputs["emb_b2"], f32)[idx]

    per_core = []
    bf = np.dtype('bfloat16') if hasattr(np, 'bfloat16') else None
    import ml_dtypes
    bf16 = ml_dtypes.bfloat16

    if dis_on:
        dm0 = dis_input[0].mean(axis=0, dtype=f32)
        ds = dm0 @ np.asarray(inputs["dis_sel_W"], f32) + np.asarray(inputs["dis_sel_b"], f32)
        dtop = _top2(ds)
        w_at_base = np.empty((2, D, D), f32)
        ab_base = np.empty((2, D), f32)       # ln1_b @ attnW_g + attn_b
        w_f1 = np.empty((2, D, 2 * D), f32)
        b_f1 = np.empty((2, 2 * D), f32)
        w_f2 = np.empty((2, 2 * D, D), f32)
        b_f2h = np.empty((2, D), f32)
        for i, idx in enumerate(dtop):
            g1 = np.asarray(inputs["dis_ln1_g"], f32)[idx]
            b1 = np.asarray(inputs["dis_ln1_b"], f32)[idx]
            aw = np.asarray(inputs["dis_attn_W"], f32)[idx]
            w_at_base[i] = g1[:, None] * aw
            ab_base[i] = b1 @ aw + np.asarray(inputs["dis_attn_b"], f32)[idx]
            g2 = np.asarray(inputs["dis_ln2_g"], f32)[idx]
            b2 = np.asarray(inputs["dis_ln2_b"], f32)[idx]
            f1 = np.asarray(inputs["dis_ff1_W"], f32)[idx]
            w_f1[i] = g2[:, None] * f1
            b_f1[i] = b2 @ f1 + np.asarray(inputs["dis_ff1_b"], f32)[idx]
            w_f2[i] = 0.5 * np.asarray(inputs["dis_ff2_W"], f32)[idx]
            b_f2h[i] = 0.5 * np.asarray(inputs["dis_ff2_b"], f32)[idx]
        w_f1_bf = w_f1.astype(bf16)
        w_f2_bf = w_f2.astype(bf16)

    w_e1_bf = w_e1.astype(bf16)

    for c in range(N_CORES):
        t_emb3 = 0.3 * (1.0 + 0.1 * torsion[c])      # [D]
        w_e2 = (w_e2_base * t_emb3[None, None, :]).astype(bf16)
        b_e2s = (b_e2_base * t_emb3[None, :]).astype(f32)

        d = {
            "x_emb": np.ascontiguousarray(emb_input[c]),
            "w_e1": w_e1_bf,
            "w_e2": w_e2,
        }
        # vec512 layout: [be1_0, be1_1, be2s_0, be2s_1, ab_0, ab_1, dsc, bf2_0, bf2_1]
        vec512 = np.zeros((9, D), f32)
        vec512[0] = b_e1[0]
        vec512[1] = b_e1[1]
        vec512[2] = b_e2s[0]
        vec512[3] = b_e2s[1]

        if dis_on:
            td05 = 0.5 * (1.0 + 0.05 * torsion[c])   # [D]
            w_at = (w_at_base * td05[None, None, :]).astype(bf16)
            vec512[4] = td05 * ab_base[0]
            vec512[5] = td05 * ab_base[1]
            vec512[6] = td05 * 0.2 / S               # multiplies delayed SUM
            vec512[7] = b_f2h[0]
            vec512[8] = b_f2h[1]
            vec1024 = np.stack([b_f1[0], b_f1[1]]).astype(f32)
            d.update({
                "x_dis": np.ascontiguousarray(dis_input[c]),
                "w_at": w_at,
                "w_f1": w_f1_bf,
                "w_f2": w_f2_bf,
                "vec1024": vec1024,
            })
        d["vec512"] = vec512
        per_core.append(d)
    return per_core, dis_on


# ----------------------------------------------------------------------------
# Device program
# ----------------------------------------------------------------------------

def _build_module(T, dis_on):
    import concourse.bass as bass
    import concourse.mybir as mybir
    import concourse.tile as tile
    from concourse import bacc
    from contextlib import ExitStack

    f32 = mybir.dt.float32
    bf16 = mybir.dt.bfloat16
    i32 = mybir.dt.int32
    Alu = mybir.AluOpType
    Act = mybir.ActivationFunctionType

    NCH = T // TC
    GRP = min(4, NCH)
    NG = NCH // GRP
    TGRP = min(2048, T)      # input transpose-load token group

    nc = bacc.Bacc("TRN2", target_bir_lowering=False, debug=False,
                   num_devices=N_CORES)

    x_emb = nc.dram_tensor("x_emb", [T, D], f32, kind="ExternalInput")
    w_e1 = nc.dram_tensor("w_e1", [2, D, D], bf16, kind="ExternalInput")
    w_e2 = nc.dram_tensor("w_e2", [2, D, D], bf16, kind="ExternalInput")
    vec512 = nc.dram_tensor("vec512", [9, D], f32, kind="ExternalInput")
    y_emb = nc.dram_tensor("y_emb", [T, D], f32, kind="ExternalOutput")
    s_tok_e = nc.dram_tensor("s_tok_e", [T, D], bf16, kind="Internal")
    s_feat_e = nc.dram_tensor("s_feat_e", [D, T], bf16, kind="Internal")
    if dis_on:
        x_dis = nc.dram_tensor("x_dis", [T, D], f32, kind="ExternalInput")
        w_at = nc.dram_tensor("w_at", [2, D, D], bf16, kind="ExternalInput")
        w_f1 = nc.dram_tensor("w_f1", [2, D, 2 * D], bf16, kind="ExternalInput")
        w_f2 = nc.dram_tensor("w_f2", [2, 2 * D, D], bf16, kind="ExternalInput")
        vec1024 = nc.dram_tensor("vec1024", [2, 2 * D], f32, kind="ExternalInput")
        y_dis = nc.dram_tensor("y_dis", [T, D], f32, kind="ExternalOutput")
        s_tok_d = nc.dram_tensor("s_tok_d", [T, D], bf16, kind="Internal")
        s_feat_d = nc.dram_tensor("s_feat_d", [D, T], bf16, kind="Internal")

    with tile.TileContext(nc) as tc, ExitStack() as ctx:
        sb = ctx.enter_context(tc.tile_pool(name="sb", bufs=1))
        psum = ctx.enter_context(tc.tile_pool(name="psum", bufs=1, space="PSUM"))

        # ---- constants
        ones_sc = sb.tile([P, P], bf16, tag="ones_sc", name="ones_sc")
        nc.vector.memset(ones_sc, 1.0 / D)
        ones_row = sb.tile([1, P], bf16, tag="ones_row", name="ones_row")
        nc.vector.memset(ones_row, 1.0)
        magic = sb.tile([P, TC], i32, tag="magic", name="magic")
        nc.vector.memset(magic, 0x5f3759df)
        eps_t = sb.tile([P, 1], f32, tag="eps_t", name="eps_t")
        nc.vector.memset(eps_t, EPS)

        # ---- small vectors [128, 9, 4]
        v512 = sb.tile([P, 9, KB], f32, tag="v512", name="v512")
        nc.sync.dma_start(out=v512, in_=vec512[:, :].rearrange("v (a p) -> p v a", p=P))

        def vec_ap(v, mb):
            return v512[:, v, mb:mb + 1]

        if dis_on:
            v1024 = sb.tile([P, 2, 8], f32, tag="v1024", name="v1024")
            nc.sync.dma_start(out=v1024, in_=vec1024[:, :].rearrange("v (a p) -> p v a", p=P))

        # ---- weights (feature-major lhsT layout [P, kb, m])
        def load_w(handle, i, kblocks, mtot, tag):
            t = sb.tile([P, kblocks, mtot], bf16, tag=tag, name=tag)
            nc.sync.dma_start(
                out=t, in_=handle[i:i + 1].rearrange("o (a p) m -> p (o a) m", p=P))
            return t

        we1 = [load_w(w_e1, i, KB, D, f"we1_{i}") for i in range(2)]
        we2 = [load_w(w_e2, i, KB, D, f"we2_{i}") for i in range(2)]
        if dis_on:
            wat = [load_w(w_at, i, KB, D, f"wat_{i}") for i in range(2)]
            wf1 = [load_w(w_f1, i, KB, 2 * D, f"wf1_{i}") for i in range(2)]
            wf2 = [load_w(w_f2, i, 2 * KB, D, f"wf2_{i}") for i in range(2)]

        # ---- residual tiles (feature-major, reused emb -> dis via tag cycling)
        def alloc_h(which):
            return [sb.tile([P, T], bf16, tag=f"h{pb}", name=f"h_{which}{pb}")
                    for pb in range(KB)]

        def load_stream(h, x_h, s_tok):
            ng = T // TGRP
            for g in range(ng):
                sl = slice(g * TGRP, (g + 1) * TGRP)
                nc.gpsimd.dma_start(out=s_tok[sl, :], in_=x_h[sl, :])  # f32 -> bf16
            for pb in range(KB):
                for g in range(ng):
                    sl = slice(g * TGRP, (g + 1) * TGRP)
                    nc.sync.dma_start(out=h[pb][:, sl],
                                      in_=s_tok[sl, P * pb:P * (pb + 1)],
                                      transpose=True)

        def store_stream(h, s_feat, y_h):
            for pb in range(KB):
                nc.sync.dma_start(out=s_feat[P * pb:P * (pb + 1), :], in_=h[pb][:, :])
            ng = T // TC
            for g in range(ng):
                ot = sb.tile([P, KB, D], bf16, tag="ot", bufs=1, name="ot")
                for a in range(KB):
                    t0 = g * TC + a * P
                    nc.sync.dma_start(out=ot[:, a, :],
                                      in_=s_feat[:, t0:t0 + P], transpose=True)
                nc.gpsimd.dma_start(
                    out=y_h[g * TC:(g + 1) * TC, :].rearrange("(a p) d -> p a d", p=P),
                    in_=ot)  # bf16 -> f32

        # ---- LN stats machinery
        def newton_rsqrt(st):
            """st: [P, TC] f32 (var+eps, chunk j of the group replicated on
            partitions 32j..32j+31) -> [P, TC] bf16 rstd."""
            sh = sb.tile([P, TC], i32, tag="nsh", bufs=1, name="nsh")
            nc.vector.tensor_scalar(out=sh, in0=st.bitcast(i32), scalar1=1,
                                    scalar2=None, op0=Alu.arith_shift_right)
            y = sb.tile([P, TC], f32, tag="ny", bufs=1, name="ny")
            nc.vector.tensor_sub(y.bitcast(i32), magic, sh)
            vh = sb.tile([P, TC], f32, tag="nvh", bufs=1, name="nvh")
            nc.vector.tensor_scalar(out=vh, in0=st, scalar1=-0.5, scalar2=None,
                                    op0=Alu.mult)
            t0 = sb.tile([P, TC], f32, tag="nt0", bufs=1, name="nt0")
            t1 = sb.tile([P, TC], f32, tag="nt1", bufs=1, name="nt1")
            rs = sb.tile([P, TC], bf16, tag="nrs", bufs=2, name="nrs")
            for it in range(NEWTON_ITERS):
                nc.vector.tensor_mul(t0, y, y)
                nc.vector.tensor_mul(t1, t0, vh)
                nc.vector.tensor_scalar(out=t1, in0=t1, scalar1=1.5, scalar2=None,
                                        op0=Alu.add)
                nc.vector.tensor_mul(rs if it == NEWTON_ITERS - 1 else y, y, t1)
            return rs

        class LNPhase:
            """One LN + its consumer (matmuls/activations/residual)."""

            def __init__(self, h, main_fn, name):
                self.h = h          # list of KB residual tiles (stats input)
                self.main_fn = main_fn
                self.name = name
                self.rc = {}
                self.rz = {}

            def stats_chunk(self, k):
                h = self.h
                j = k % GRP
                if j == 0:
                    self._st = sb.tile([P, TC], f32, tag="st", bufs=2, name="st")
                st = self._st
                ck = slice(k * TC, (k + 1) * TC)
                m_ps = psum.tile([P, TC], f32, tag="stats_ps", bufs=3, name="m_ps")
                for kb in range(KB):
                    nc.tensor.matmul(m_ps, ones_sc, h[kb][:, ck],
                                     start=kb == 0, stop=kb == KB - 1)
                m_b = sb.tile([P, TC], bf16, tag="m_b", bufs=2, name="m_b")
                nc.scalar.copy(m_b, m_ps)
                rcs = []
                v_ps = psum.tile([P, TC], f32, tag="stats_ps", bufs=3, name="v_ps")
                for kb in range(KB):
                    rc = sb.tile([P, TC], bf16, tag=f"rc{kb}", bufs=GRP + 1,
                                 name=f"rc{kb}")
                    nc.vector.tensor_sub(rc, h[kb][:, ck], m_b)
                    rcs.append(rc)
                    x2 = sb.tile([P, TC], bf16, tag="x2", bufs=2, name="x2")
                    nc.scalar.square(x2, rc)
                    nc.tensor.matmul(v_ps, ones_sc, x2,
                                     start=kb == 0, stop=kb == KB - 1)
                self.rc[k] = rcs
                W = P // GRP
                nc.scalar.activation(st[W * j:W * (j + 1), :], v_ps[0:W, :],
                                     Act.Identity, bias=eps_t[0:W, 0:1],
                                     scale=1.0)
                if j == GRP - 1:
                    rs = newton_rsqrt(st)
                    for jj in range(GRP):
                        kk = k - (GRP - 1) + jj
                        rz = sb.tile([1, TC], bf16, tag="rz", bufs=GRP + 2,
                                     name="rz")
                        nc.sync.dma_start(out=rz, in_=rs[W * jj:W * jj + 1, :])
                        self.rz[kk] = rz

            def main_chunk(self, k):
                ck = slice(k * TC, (k + 1) * TC)
                rb_ps = psum.tile([P, TC], f32, tag="stats_ps", bufs=3,
                                  name="rb_ps")
                nc.tensor.matmul(rb_ps, ones_row, self.rz.pop(k),
                                 start=True, stop=True)
                rstd_b = sb.tile([P, TC], bf16, tag="rstd_b", bufs=2,
                                 name="rstd_b")
                nc.scalar.copy(rstd_b, rb_ps)
                rcs = self.rc.pop(k)
                xh = []
                for kb in range(KB):
                    t = sb.tile([P, TC], bf16, tag=f"xh{kb}", bufs=2,
                                name=f"xh{kb}")
                    nc.vector.tensor_mul(t, rcs[kb], rstd_b)
                    xh.append(t)
                self.main_fn(k, ck, xh)

        Add = Alu.add

        def emb_main(i):
            def fn(k, ck, xh):
                h = hE
                u_list = []
                for mb in range(KB):
                    u_ps = psum.tile([P, TC], f32, tag="mm_ps", bufs=5, name="u_ps")
                    for kb in range(KB):
                        nc.tensor.matmul(u_ps, we1[i][:, kb, P * mb:P * (mb + 1)],
                                         xh[kb], start=kb == 0, stop=kb == KB - 1)
                    u_list.append(u_ps)
                a_list = []
                for mb in range(KB):
                    a = sb.tile([P, TC], bf16, tag=f"a{mb}", bufs=2, name=f"a{mb}")
                    nc.scalar.activation(a, u_list[mb], Act.Tanh,
                                         bias=vec_ap(i, mb), scale=1.0)
                    a_list.append(a)
                for mb in range(KB):
                    v_ps = psum.tile([P, TC], f32, tag="mm_ps", bufs=5, name="v_ps2")
                    for kb in range(KB):
                        nc.tensor.matmul(v_ps, we2[i][:, kb, P * mb:P * (mb + 1)],
                                         a_list[kb], start=kb == 0, stop=kb == KB - 1)
                    nc.vector.scalar_tensor_tensor(
                        out=h[mb][:, ck], in0=v_ps, scalar=vec_ap(2 + i, mb),
                        in1=h[mb][:, ck], op0=Add, op1=Add)
            return fn

        def dis_attn_main(i):
            def fn(k, ck, xh):
                h = hD
                for mb in range(KB):
                    u_ps = psum.tile([P, TC], f32, tag="mm_ps", bufs=5, name="ua_ps")
                    for kb in range(KB):
                        nc.tensor.matmul(u_ps, wat[i][:, kb, P * mb:P * (mb + 1)],
                                         xh[kb], start=kb == 0, stop=kb == KB - 1)
                    nc.vector.scalar_tensor_tensor(
                        out=h[mb][:, ck], in0=u_ps, scalar=bias_dis[i][:, mb:mb + 1],
                        in1=h[mb][:, ck], op0=Add, op1=Add)
            return fn

        def dis_ff_main(i):
            def fn(k, ck, xh):
                h = hD
                g_list = []
                for mb8 in range(2 * KB):
                    g_ps = psum.tile([P, TC], f32, tag="mm_ps", bufs=5, name="g_ps")
                    for kb in range(KB):
                        nc.tensor.matmul(g_ps, wf1[i][:, kb, P * mb8:P * (mb8 + 1)],
                                         xh[kb], start=kb == 0, stop=kb == KB - 1)
                    gt = sb.tile([P, TC], bf16, tag=f"g{mb8}", bufs=2, name=f"g{mb8}")
                    nc.scalar.activation(gt, g_ps, getattr(Act, GELU_FUNC_NAME),
                                         bias=v1024[:, i, mb8:mb8 + 1], scale=1.0)
                    g_list.append(gt)
                for mb in range(KB):
                    h2_ps = psum.tile([P, TC], f32, tag="mm_ps", bufs=5, name="h2_ps")
                    for kb8 in range(2 * KB):
                        nc.tensor.matmul(h2_ps, wf2[i][:, kb8, P * mb:P * (mb + 1)],
                                         g_list[kb8], start=kb8 == 0,
                                         stop=kb8 == 2 * KB - 1)
                    nc.vector.scalar_tensor_tensor(
                        out=h[mb][:, ck], in0=h2_ps, scalar=vec_ap(7 + i, mb),
                        in1=h[mb][:, ck], op0=Add, op1=Add)
            return fn

        # ---- emb stream
        hE = alloc_h("e")
        load_stream(hE, x_emb, s_tok_e)
        phases = [LNPhase(hE, emb_main(0), "e0"), LNPhase(hE, emb_main(1), "e1")]

        if dis_on:
            bias_dis = []

        def emit(phs):
            # software-pipelined emission at chunk granularity: stats run
            # GRP chunks ahead of main (enough for the group-batched Newton
            # to complete before the first main of the group).  Requires
            # NCH > GRP so cross-phase stats never precede the main that
            # produces their input; otherwise fall back to serial groups.
            sq = [(ph, k) for ph in phs for k in range(NCH)]
            L = GRP
            if NCH <= L:
                for ph in phs:
                    for k in range(NCH):
                        ph.stats_chunk(k)
                    for k in range(NCH):
                        ph.main_chunk(k)
                return
            for i, (ph, k) in enumerate(sq):
                ph.stats_chunk(k)
                if i - L >= 0:
                    pj, kj = sq[i - L]
                    pj.main_chunk(kj)
            for i in range(len(sq) - L, len(sq)):
                pj, kj = sq[i]
                pj.main_chunk(kj)

        emit(phases)
        store_stream(hE, s_feat_e, y_emb)

        if dis_on:
            hD = alloc_h("d")     # same tags as hE -> reuses slots after emb done
            load_stream(hD, x_dis, s_tok_d)
            # delayed sum + per-block hm-bias vectors
            dsum = [sb.tile([P, 1], f32, tag=f"dsum{pb}", name=f"dsum{pb}")
                    for pb in range(KB)]
            for pb in range(KB):
                nc.vector.tensor_reduce(out=dsum[pb], in_=hD[pb][:, :],
                                        axis=mybir.AxisListType.X, op=Alu.add)
            for i in range(2):
                bd = sb.tile([P, KB], f32, tag=f"bias_dis{i}", name=f"bias_dis{i}")
                for mb in range(KB):
                    nc.vector.tensor_scalar(
                        out=bd[:, mb:mb + 1], in0=dsum[mb], scalar1=vec_ap(6, mb),
                        scalar2=vec_ap(4 + i, mb), op0=Alu.mult, op1=Alu.add)
                bias_dis.append(bd)

            dphases = [LNPhase(hD, dis_attn_main(0), "d0a"),
                       LNPhase(hD, dis_ff_main(0), "d0f"),
                       LNPhase(hD, dis_attn_main(1), "d1a"),
                       LNPhase(hD, dis_ff_main(1), "d1f")]
            emit(dphases)
            store_stream(hD, s_feat_d, y_dis)

    nc.compile()
    return nc


# ----------------------------------------------------------------------------
# Entry point
# ----------------------------------------------------------------------------

def _get_module(T, dis_on):
    key = (T, dis_on, GELU_FUNC_NAME)
    if key not in _MODULE_CACHE:
        _MODULE_CACHE[key] = _build_module(T, dis_on)
    return _MODULE_CACHE[key]


LAST_EXEC_TIME_NS = None
TRACE = False


def kernel(**inputs):
    global LAST_EXEC_TIME_NS
    from concourse.bass_utils import run_bass_kernel_spmd

    per_core, dis_on = _prep_host(inputs)
    nc = _get_module(S, dis_on)

    res = run_bass_kernel_spmd(nc, per_core, core_ids=list(range(N_CORES)),
                               trace=TRACE)
    LAST_EXEC_TIME_NS = res.exec_time_ns

    emb = np.stack([res.results[c]["y_emb"] for c in range(N_CORES)])
    if dis_on:
        dis = np.stack([res.results[c]["y_dis"] for c in range(N_CORES)])
    else:
        dis = None
    return emb, dis


# revision 31
# speedup vs baseline: 112.4161x; 112.4161x over previous
"""Trainium2 Bass kernel for nn_DevLayer_12627203850761 (moe_routing).

Strategy:
  - Batch-parallel across 8 NeuronCores: core c processes batch element c
    of both streams (emb + dis). No collectives needed (routing top-2 and
    per-block weight gather/folding done host-side; `delayed` is a
    per-batch mean so it is core-local).
  - On device, activations live feature-major ([D partitions, T free]) in
    bf16; all matmuls use the weights as the stationary lhsT operand.
  - LayerNorm mean/var are computed with ones-matmuls on the PE (free on
    the bottleneck-adjacent engines), rstd via a bit-hack+Newton rsqrt on
    the vector engine (ACT Rsqrt is banned), broadcast back over
    partitions with a K=1 ones-matmul.
  - LN gamma/beta, biases, torsion factors and the 0.5/0.3 residual
    scales are folded into the weights / per-feature bias vectors on the
    host, so the device only does: stats, center, scale, matmul,
    ACT(tanh/gelu) with per-partition bias, and one fused
    scalar_tensor_tensor per residual add.
  - Layout changes (token-major f32 DRAM <-> feature-major bf16 SBUF) are
    done purely with DMA: SWDGE cast-DMA (f32<->bf16) + HWDGE xbar
    transpose (2-byte dtype).  NOTE: all xbar-transpose DMAs and
    SBUF->SBUF copies must stay on the SAME HWDGE ring (nc.sync) — running
    them concurrently on both rings trips the documented DMA-transpose ||
    SBUF->SBUF hardware hazard and silently corrupts data (observed: rel
    err 0.34 with input transposes moved to the ACT ring).
"""

import sys
import numpy as np

if '/opt/trn_rl_repo' not in sys.path:
    sys.path.insert(0, '/opt/trn_rl_repo')

B, S, D, NB = 8, 8192, 512, 16
P = 128
KB = D // P            # 4 feature blocks
TC = 512               # token chunk (PSUM free dim)
EPS = 1e-5
N_CORES = 8
GELU_FUNC_NAME = "Gelu"   # CoreSim has no Gelu; sim tests swap in "Tanh"

# tuning knobs (consulted at build time; include in cache key)
CFG = {
    "stats_ps_bufs": 3,
    "mm_ps_bufs": 5,
    "rc_bufs_extra": 1,      # rc bufs = GRP + this
    "newton_iters": 1,
    "lookahead_extra": 0,    # L = GRP + this
}

_MODULE_CACHE = {}


# ----------------------------------------------------------------------------
# Host-side routing + weight folding
# ----------------------------------------------------------------------------

def _top2(scores_row):
    # jax.lax.top_k: descending values, ties -> lower index first
    idx = np.lexsort((np.arange(scores_row.shape[0]), -scores_row))
    return int(idx[0]), int(idx[1])


def _prep_host(inputs):
    """Compute routing and folded per-core device inputs."""
    f32 = np.float32
    emb_input = np.asarray(inputs["emb_input"], f32)
    dis_input = np.asarray(inputs["dis_input"], f32)
    torsion = np.asarray(inputs["torsion"], f32)
    dis_on = bool(int(inputs["dis_unlocked"]))

    # ---- routing (sigmoid is monotonic -> top_k on logits)
    m0 = emb_input[0].mean(axis=0, dtype=f32)                       # [D]
    es = m0 @ np.asarray(inputs["emb_sel_W"], f32) + np.asarray(inputs["emb_sel_b"], f32)
    etop = _top2(es)

    # ---- emb folded weights (shared across cores)
    w_e1 = np.empty((2, D, D), f32)
    b_e1 = np.empty((2, D), f32)
    w_e2_base = np.empty((2, D, D), f32)
    b_e2_base = np.empty((2, D), f32)
    for i, idx in enumerate(etop):
        g = np.asarray(inputs["emb_ln_g"], f32)[idx]
        b = np.asarray(inputs["emb_ln_b"], f32)[idx]
        w1 = np.asarray(inputs["emb_w1"], f32)[idx]
        w_e1[i] = g[:, None] * w1
        b_e1[i] = b @ w1 + np.asarray(inputs["emb_b1"], f32)[idx]
        w_e2_base[i] = np.asarray(inputs["emb_w2"], f32)[idx]
        b_e2_base[i] = np.asarray(inputs["emb_b2"], f32)[idx]

    per_core = []
    bf = np.dtype('bfloat16') if hasattr(np, 'bfloat16') else None
    import ml_dtypes
    bf16 = ml_dtypes.bfloat16

    if dis_on:
        dm0 = dis_input[0].mean(axis=0, dtype=f32)
        ds = dm0 @ np.asarray(inputs["dis_sel_W"], f32) + np.asarray(inputs["dis_sel_b"], f32)
        dtop = _top2(ds)
        w_at_base = np.empty((2, D, D), f32)
        ab_base = np.empty((2, D), f32)       # ln1_b @ attnW_g + attn_b
        w_f1 = np.empty((2, D, 2 * D), f32)
        b_f1 = np.empty((2, 2 * D), f32)
        w_f2 = np.empty((2, 2 * D, D), f32)
        b_f2h = np.empty((2, D), f32)
        for i, idx in enumerate(dtop):
            g1 = np.asarray(inputs["dis_ln1_g"], f32)[idx]
            b1 = np.asarray(inputs["dis_ln1_b"], f32)[idx]
            aw = np.asarray(inputs["dis_attn_W"], f32)[idx]
            w_at_base[i] = g1[:, None] * aw
            ab_base[i] = b1 @ aw + np.asarray(inputs["dis_attn_b"], f32)[idx]
            g2 = np.asarray(inputs["dis_ln2_g"], f32)[idx]
            b2 = np.asarray(inputs["dis_ln2_b"], f32)[idx]
            f1 = np.asarray(inputs["dis_ff1_W"], f32)[idx]
            w_f1[i] = g2[:, None] * f1
            b_f1[i] = b2 @ f1 + np.asarray(inputs["dis_ff1_b"], f32)[idx]
            w_f2[i] = 0.5 * np.asarray(inputs["dis_ff2_W"], f32)[idx]
            b_f2h[i] = 0.5 * np.asarray(inputs["dis_ff2_b"], f32)[idx]
        w_f1_bf = w_f1.astype(bf16)
        w_f2_bf = w_f2.astype(bf16)

    w_e1_bf = w_e1.astype(bf16)

    for c in range(N_CORES):
        t_emb3 = 0.3 * (1.0 + 0.1 * torsion[c])      # [D]
        w_e2 = (w_e2_base * t_emb3[None, None, :]).astype(bf16)
        b_e2s = (b_e2_base * t_emb3[None, :]).astype(f32)

        d = {
            "x_emb": np.ascontiguousarray(emb_input[c]),
            "w_e1": w_e1_bf,
            "w_e2": w_e2,
        }
        # vec512 layout: [be1_0, be1_1, be2s_0, be2s_1, ab_0, ab_1, dsc, bf2_0, bf2_1]
        vec512 = np.zeros((9, D), f32)
        vec512[0] = b_e1[0]
        vec512[1] = b_e1[1]
        vec512[2] = b_e2s[0]
        vec512[3] = b_e2s[1]

        if dis_on:
            td05 = 0.5 * (1.0 + 0.05 * torsion[c])   # [D]
            w_at = (w_at_base * td05[None, None, :]).astype(bf16)
            vec512[4] = td05 * ab_base[0]
            vec512[5] = td05 * ab_base[1]
            vec512[6] = td05 * 0.2 / S               # multiplies delayed SUM
            vec512[7] = b_f2h[0]
            vec512[8] = b_f2h[1]
            vec1024 = np.stack([b_f1[0], b_f1[1]]).astype(f32)
            d.update({
                "x_dis": np.ascontiguousarray(dis_input[c]),
                "w_at": w_at,
                "w_f1": w_f1_bf,
                "w_f2": w_f2_bf,
                "vec1024": vec1024,
            })
        d["vec512"] = vec512
        per_core.append(d)
    return per_core, dis_on


# ----------------------------------------------------------------------------
# Device program
# ----------------------------------------------------------------------------

def _build_module(T, dis_on):
    import concourse.bass as bass
    import concourse.mybir as mybir
    import concourse.tile as tile
    from concourse import bacc
    from contextlib import ExitStack

    f32 = mybir.dt.float32
    bf16 = mybir.dt.bfloat16
    i32 = mybir.dt.int32
    Alu = mybir.AluOpType
    Act = mybir.ActivationFunctionType

    NCH = T // TC
    GRP = min(4, NCH)
    NG = NCH // GRP
    # graduated input-group sizes: small first groups so compute starts early
    GS = []
    rem = T
    for sz in (512, 512, 1024):
        if rem > 2048 and sz <= rem:
            GS.append(sz)
            rem -= sz
    while rem > 0:
        sz = min(2048, rem)
        GS.append(sz)
        rem -= sz
    GOFF = [0]
    for sz in GS:
        GOFF.append(GOFF[-1] + sz)

    nc = bacc.Bacc("TRN2", target_bir_lowering=False, debug=False,
                   num_devices=N_CORES)

    x_emb = nc.dram_tensor("x_emb", [T, D], f32, kind="ExternalInput")
    w_e1 = nc.dram_tensor("w_e1", [2, D, D], bf16, kind="ExternalInput")
    w_e2 = nc.dram_tensor("w_e2", [2, D, D], bf16, kind="ExternalInput")
    vec512 = nc.dram_tensor("vec512", [9, D], f32, kind="ExternalInput")
    y_emb = nc.dram_tensor("y_emb", [T, D], f32, kind="ExternalOutput")
    s_tok_e = nc.dram_tensor("s_tok_e", [T, D], bf16, kind="Internal")
    s_feat_e = nc.dram_tensor("s_feat_e", [D, T], bf16, kind="Internal")
    if dis_on:
        x_dis = nc.dram_tensor("x_dis", [T, D], f32, kind="ExternalInput")
        w_at = nc.dram_tensor("w_at", [2, D, D], bf16, kind="ExternalInput")
        w_f1 = nc.dram_tensor("w_f1", [2, D, 2 * D], bf16, kind="ExternalInput")
        w_f2 = nc.dram_tensor("w_f2", [2, 2 * D, D], bf16, kind="ExternalInput")
        vec1024 = nc.dram_tensor("vec1024", [2, 2 * D], f32, kind="ExternalInput")
        y_dis = nc.dram_tensor("y_dis", [T, D], f32, kind="ExternalOutput")
        s_tok_d = nc.dram_tensor("s_tok_d", [T, D], bf16, kind="Internal")
        s_feat_d = nc.dram_tensor("s_feat_d", [D, T], bf16, kind="Internal")

    with tile.TileContext(nc) as tc, ExitStack() as ctx:
        sb = ctx.enter_context(tc.tile_pool(name="sb", bufs=1))
        psum = ctx.enter_context(tc.tile_pool(name="psum", bufs=1, space="PSUM"))

        # ---- constants
        ones_sc = sb.tile([P, P], bf16, tag="ones_sc", name="ones_sc")
        nc.vector.memset(ones_sc, 1.0 / D)
        ones_row = sb.tile([1, P], bf16, tag="ones_row", name="ones_row")
        nc.vector.memset(ones_row, 1.0)
        magic = sb.tile([P, TC], i32, tag="magic", name="magic")
        nc.vector.memset(magic, 0x5f3759df)
        eps_t = sb.tile([P, 1], f32, tag="eps_t", name="eps_t")
        nc.vector.memset(eps_t, EPS)

        # ---- small vectors [128, 9, 4]
        v512 = sb.tile([P, 9, KB], f32, tag="v512", name="v512")
        nc.sync.dma_start(out=v512, in_=vec512[:, :].rearrange("v (a p) -> p v a", p=P))

        def vec_ap(v, mb):
            return v512[:, v, mb:mb + 1]

        if dis_on:
            v1024 = sb.tile([P, 2, 8], f32, tag="v1024", name="v1024")
            nc.sync.dma_start(out=v1024, in_=vec1024[:, :].rearrange("v (a p) -> p v a", p=P))

        # ---- weights (feature-major lhsT layout [P, kb, m])
        def load_w(handle, i, kblocks, mtot, tag, bufs=1):
            t = sb.tile([P, kblocks, mtot], bf16, tag=tag, name=f"{tag}_ld", bufs=bufs)
            nc.sync.dma_start(
                out=t, in_=handle[i:i + 1].rearrange("o (a p) m -> p (o a) m", p=P))
            return t

        we1 = [load_w(w_e1, i, KB, D, f"wA{i}", bufs=1) for i in range(2)]
        we2 = [load_w(w_e2, i, KB, D, f"wA{2 + i}", bufs=1) for i in range(2)]
        if dis_on:
            wf1 = [load_w(w_f1, i, KB, 2 * D, f"wf1_{i}") for i in range(2)]
            wf2 = [load_w(w_f2, i, 2 * KB, D, f"wf2_{i}") for i in range(2)]
            wat = None   # loaded later into the wA slots (after emb finishes)

        # ---- residual stream: per-(pblock, token-group) tiles so slot reuse
        # (emb -> dis) and load/compute overlap happen at group granularity
        NGRP = len(GS)
        import bisect

        def group_of_chunk(k):
            g = bisect.bisect_right(GOFF, k * TC) - 1
            return g, k * TC - GOFF[g]

        LAST_CHUNK_OF_GROUP = {(GOFF[g + 1] // TC) - 1: g for g in range(NGRP)}

        class HStream:
            def __init__(self, which):
                self.which = which
                self.groups = [[None] * NGRP for _ in range(KB)]

            def alloc_group(self, g):
                for pb in range(KB):
                    self.groups[pb][g] = sb.tile(
                        [P, GS[g]], bf16, tag=f"h{pb}g{g}",
                        name=f"h_{self.which}{pb}g{g}")

            def ap(self, pb, k):
                g, off = group_of_chunk(k)
                t = self.groups[pb][g]
                return t[:, off:off + TC]

        def load_group(hs, x_h, s_tok, g):
            sl = slice(GOFF[g], GOFF[g + 1])
            nc.gpsimd.dma_start(out=s_tok[sl, :], in_=x_h[sl, :])  # f32 -> bf16
            for pb in range(KB):
                nc.sync.dma_start(out=hs.groups[pb][g],
                                  in_=s_tok[sl, P * pb:P * (pb + 1)],
                                  transpose=True)

        def store_chunk(hs, s_feat, y_h, k, ck):
            for pb in range(KB):
                nc.sync.dma_start(out=s_feat[P * pb:P * (pb + 1), ck],
                                  in_=hs.ap(pb, k))
            ot = sb.tile([P, KB, D], bf16, tag="ot", bufs=1, name="ot")
            for a in range(KB):
                t0 = k * TC + a * P
                nc.sync.dma_start(out=ot[:, a, :],
                                  in_=s_feat[:, t0:t0 + P], transpose=True)
            nc.gpsimd.dma_start(
                out=y_h[k * TC:(k + 1) * TC, :].rearrange("(a p) d -> p a d", p=P),
                in_=ot)  # bf16 -> f32

        # ---- LN stats machinery
        def newton_rsqrt(st):
            """st: [P, TC] f32 (var+eps, chunk j of the group replicated on
            partitions Wj..W(j+1)) -> [P, TC] bf16 rstd."""
            sh = sb.tile([P, TC], i32, tag="nsh", bufs=1, name="nsh")
            nc.vector.tensor_scalar(out=sh, in0=st.bitcast(i32), scalar1=1,
                                    scalar2=None, op0=Alu.arith_shift_right)
            y = sb.tile([P, TC], f32, tag="ny", bufs=1, name="ny")
            nc.vector.tensor_sub(y.bitcast(i32), magic, sh)
            vh = sb.tile([P, TC], f32, tag="nvh", bufs=1, name="nvh")
            nc.vector.tensor_scalar(out=vh, in0=st, scalar1=-0.5, scalar2=None,
                                    op0=Alu.mult)
            t0 = sb.tile([P, TC], f32, tag="nt0", bufs=1, name="nt0")
            t1 = sb.tile([P, TC], f32, tag="nt1", bufs=1, name="nt1")
            rs = sb.tile([P, TC], bf16, tag="nrs", bufs=2, name="nrs")
            for it in range(CFG["newton_iters"]):
                nc.vector.tensor_mul(t0, y, y)
                nc.vector.tensor_mul(t1, t0, vh)
                nc.vector.tensor_scalar(out=t1, in0=t1, scalar1=1.5, scalar2=None,
                                        op0=Alu.add)
                nc.vector.tensor_mul(rs if it == CFG["newton_iters"] - 1 else y, y, t1)
            return rs

        class LNPhase:
            """One LN + its consumer (matmuls/activations/residual)."""

            def __init__(self, h, main_fn, name, after_chunk=None,
                         sq_dve=False):
                self.h = h          # HStream (stats input / residual)
                self.main_fn = main_fn
                self.name = name
                self.after_chunk = after_chunk
                self.sq_dve = sq_dve
                self.rc = {}
                self.rz = {}

            def stats_chunk(self, k):
                h = self.h
                j = k % GRP
                if j == 0:
                    self._st = sb.tile([P, TC], f32, tag="st", bufs=2, name="st")
                st = self._st
                m_ps = psum.tile([P, TC], f32, tag="stats_ps",
                                 bufs=CFG["stats_ps_bufs"], name="m_ps")
                for kb in range(KB):
                    nc.tensor.matmul(m_ps, ones_sc, h.ap(kb, k),
                                     start=kb == 0, stop=kb == KB - 1)
                m_b = sb.tile([P, TC], bf16, tag="m_b", bufs=2, name="m_b")
                nc.scalar.copy(m_b, m_ps)
                rcs = []
                v_ps = psum.tile([P, TC], f32, tag="stats_ps",
                                 bufs=CFG["stats_ps_bufs"], name="v_ps")
                for kb in range(KB):
                    rc = sb.tile([P, TC], bf16, tag=f"rc{kb}",
                                 bufs=GRP + CFG["rc_bufs_extra"], name=f"rc{kb}")
                    nc.vector.tensor_sub(rc, h.ap(kb, k), m_b)
                    rcs.append(rc)
                    x2 = sb.tile([P, TC], bf16, tag="x2", bufs=2, name="x2")
                    if self.sq_dve:
                        nc.vector.tensor_mul(x2, rc, rc)
                    else:
                        nc.scalar.square(x2, rc)
                    nc.tensor.matmul(v_ps, ones_sc, x2,
                                     start=kb == 0, stop=kb == KB - 1)
                self.rc[k] = rcs
                W = P // GRP
                nc.scalar.activation(st[W * j:W * (j + 1), :], v_ps[0:W, :],
                                     Act.Identity, bias=eps_t[0:W, 0:1],
                                     scale=1.0)
                if j == GRP - 1:
                    rs = newton_rsqrt(st)
                    for jj in range(GRP):
                        kk = k - (GRP - 1) + jj
                        rz = sb.tile([1, TC], bf16, tag="rz", bufs=GRP + 2,
                                     name="rz")
                        nc.sync.dma_start(out=rz, in_=rs[W * jj:W * jj + 1, :])
                        self.rz[kk] = rz

            def main_chunk(self, k):
                ck = slice(k * TC, (k + 1) * TC)
                rb_ps = psum.tile([P, TC], f32, tag="stats_ps",
                                  bufs=CFG["stats_ps_bufs"], name="rb_ps")
                nc.tensor.matmul(rb_ps, ones_row, self.rz.pop(k),
                                 start=True, stop=True)
                rstd_b = sb.tile([P, TC], bf16, tag="rstd_b", bufs=2,
                                 name="rstd_b")
                nc.scalar.copy(rstd_b, rb_ps)
                rcs = self.rc.pop(k)
                xh = []
                for kb in range(KB):
                    t = sb.tile([P, TC], bf16, tag=f"xh{kb}", bufs=2,
                                name=f"xh{kb}")
                    nc.vector.tensor_mul(t, rcs[kb], rstd_b)
                    xh.append(t)
                self.main_fn(k, ck, xh)
                if self.after_chunk is not None:
                    self.after_chunk(k, ck)

        Add = Alu.add

        def emb_main(i):
            def fn(k, ck, xh):
                u_list = []
                for mb in range(KB):
                    u_ps = psum.tile([P, TC], f32, tag="mm_ps",
                                     bufs=CFG["mm_ps_bufs"], name="u_ps")
                    for kb in range(KB):
                        nc.tensor.matmul(u_ps, we1[i][:, kb, P * mb:P * (mb + 1)],
                                         xh[kb], start=kb == 0, stop=kb == KB - 1)
                    u_list.append(u_ps)
                a_list = []
                for mb in range(KB):
                    a = sb.tile([P, TC], bf16, tag=f"a{mb}", bufs=2, name=f"a{mb}")
                    nc.scalar.activation(a, u_list[mb], Act.Tanh,
                                         bias=vec_ap(i, mb), scale=1.0)
                    a_list.append(a)
                for mb in range(KB):
                    v_ps = psum.tile([P, TC], f32, tag="mm_ps",
                                     bufs=CFG["mm_ps_bufs"], name="v_ps2")
                    for kb in range(KB):
                        nc.tensor.matmul(v_ps, we2[i][:, kb, P * mb:P * (mb + 1)],
                                         a_list[kb], start=kb == 0, stop=kb == KB - 1)
                    nc.vector.scalar_tensor_tensor(
                        out=hE.ap(mb, k), in0=v_ps, scalar=vec_ap(2 + i, mb),
                        in1=hE.ap(mb, k), op0=Add, op1=Add)
            return fn

        def dis_attn_main(i):
            def fn(k, ck, xh):
                for mb in range(KB):
                    u_ps = psum.tile([P, TC], f32, tag="mm_ps",
                                     bufs=CFG["mm_ps_bufs"], name="ua_ps")
                    for kb in range(KB):
                        nc.tensor.matmul(u_ps, wat[i][:, kb, P * mb:P * (mb + 1)],
                                         xh[kb], start=kb == 0, stop=kb == KB - 1)
                    nc.vector.scalar_tensor_tensor(
                        out=hD.ap(mb, k), in0=u_ps, scalar=bias_dis[i][:, mb:mb + 1],
                        in1=hD.ap(mb, k), op0=Add, op1=Add)
            return fn

        def dis_ff_main(i):
            def fn(k, ck, xh):
                g_list = []
                for mb8 in range(2 * KB):
                    g_ps = psum.tile([P, TC], f32, tag="mm_ps",
                                     bufs=CFG["mm_ps_bufs"], name="g_ps")
                    for kb in range(KB):
                        nc.tensor.matmul(g_ps, wf1[i][:, kb, P * mb8:P * (mb8 + 1)],
                                         xh[kb], start=kb == 0, stop=kb == KB - 1)
                    gt = sb.tile([P, TC], bf16, tag=f"g{mb8}", bufs=2, name=f"g{mb8}")
                    nc.scalar.activation(gt, g_ps, getattr(Act, GELU_FUNC_NAME),
                                         bias=v1024[:, i, mb8:mb8 + 1], scale=1.0)
                    g_list.append(gt)
                for mb in range(KB):
                    h2_ps = psum.tile([P, TC], f32, tag="mm_ps",
                                      bufs=CFG["mm_ps_bufs"], name="h2_ps")
                    for kb8 in range(2 * KB):
                        nc.tensor.matmul(h2_ps, wf2[i][:, kb8, P * mb:P * (mb + 1)],
                                         g_list[kb8], start=kb8 == 0,
                                         stop=kb8 == 2 * KB - 1)
                    nc.vector.scalar_tensor_tensor(
                        out=hD.ap(mb, k), in0=h2_ps, scalar=vec_ap(7 + i, mb),
                        in1=hD.ap(mb, k), op0=Add, op1=Add)
            return fn

        # ---- streams + hooks
        hE = HStream("e")
        for g in range(NGRP):
            hE.alloc_group(g)
            load_group(hE, x_emb, s_tok_e, g)

        if dis_on:
            hD = HStream("d")
            bias_dis = []
            dsum = [sb.tile([P, NGRP], f32, tag=f"dsum{pb}", name=f"dsum{pb}")
                    for pb in range(KB)]

            def dis_prep_hook(k, ck):
                nonlocal wat
                # after emb's final phase finishes group g, reuse the slots
                # for the dis stream and do the per-group delayed-sum
                if k not in LAST_CHUNK_OF_GROUP:
                    return
                g = LAST_CHUNK_OF_GROUP[k]
                if g == 0:
                    wat = [load_w(w_at, i, KB, D, f"wA{i}") for i in range(2)]
                hD.alloc_group(g)
                load_group(hD, x_dis, s_tok_d, g)
                for pb in range(KB):
                    nc.vector.tensor_reduce(out=dsum[pb][:, g:g + 1],
                                            in_=hD.groups[pb][g],
                                            axis=mybir.AxisListType.X, op=Alu.add)
                if g == NGRP - 1:
                    dsfin = [sb.tile([P, 1], f32, tag=f"dsf{pb}", name=f"dsf{pb}")
                             for pb in range(KB)]
                    for pb in range(KB):
                        nc.vector.tensor_reduce(out=dsfin[pb], in_=dsum[pb],
                                                axis=mybir.AxisListType.X,
                                                op=Alu.add)
                    for i in range(2):
                        bd = sb.tile([P, KB], f32, tag=f"bias_dis{i}",
                                     name=f"bias_dis{i}")
                        for mb in range(KB):
                            nc.vector.tensor_scalar(
                                out=bd[:, mb:mb + 1], in0=dsfin[mb],
                                scalar1=vec_ap(6, mb), scalar2=vec_ap(4 + i, mb),
                                op0=Alu.mult, op1=Alu.add)
                        bias_dis.append(bd)

            def e1_hook(k, ck):
                store_chunk(hE, s_feat_e, y_emb, k, ck)
                dis_prep_hook(k, ck)
        else:
            def e1_hook(k, ck):
                store_chunk(hE, s_feat_e, y_emb, k, ck)

        phases = [LNPhase(hE, emb_main(0), "e0"),
                  LNPhase(hE, emb_main(1), "e1", after_chunk=e1_hook)]
        if dis_on:
            phases += [
                LNPhase(hD, dis_attn_main(0), "d0a"),
                LNPhase(hD, dis_ff_main(0), "d0f", sq_dve=True),
                LNPhase(hD, dis_attn_main(1), "d1a"),
                LNPhase(hD, dis_ff_main(1), "d1f", sq_dve=True,
                        after_chunk=lambda k, ck: store_chunk(hD, s_feat_d, y_dis, k, ck)),
            ]

        def emit(phs):
            # software-pipelined emission at chunk granularity: stats run
            # L chunks ahead of main.  Requires NCH > L so cross-phase stats
            # never precede the main that produces their input; otherwise
            # fall back to serial per-phase emission.
            sq = [(ph, k) for ph in phs for k in range(NCH)]
            L = GRP + CFG["lookahead_extra"]
            if NCH <= L:
                for ph in phs:
                    for k in range(NCH):
                        ph.stats_chunk(k)
                    for k in range(NCH):
                        ph.main_chunk(k)
                return
            for i, (ph, k) in enumerate(sq):
                ph.stats_chunk(k)
                if i - L >= 0:
                    pj, kj = sq[i - L]
                    pj.main_chunk(kj)
            for i in range(len(sq) - L, len(sq)):
                pj, kj = sq[i]
                pj.main_chunk(kj)

        emit(phases)

    nc.compile()
    return nc


# ----------------------------------------------------------------------------
# Entry point
# ----------------------------------------------------------------------------

def _get_module(T, dis_on):
    key = (T, dis_on, GELU_FUNC_NAME)
    if key not in _MODULE_CACHE:
        _MODULE_CACHE[key] = _build_module(T, dis_on)
    return _MODULE_CACHE[key]


LAST_EXEC_TIME_NS = None
TRACE = False


def kernel(**inputs):
    global LAST_EXEC_TIME_NS
    from concourse.bass_utils import run_bass_kernel_spmd

    per_core, dis_on = _prep_host(inputs)
    nc = _get_module(S, dis_on)

    res = run_bass_kernel_spmd(nc, per_core, core_ids=list(range(N_CORES)),
                               trace=TRACE)
    LAST_EXEC_TIME_NS = res.exec_time_ns

    emb = np.stack([res.results[c]["y_emb"] for c in range(N_CORES)])
    if dis_on:
        dis = np.stack([res.results[c]["y_dis"] for c in range(N_CORES)])
    else:
        dis = None
    return emb, dis


# revision 32
# speedup vs baseline: 113.3277x; 1.0081x over previous
"""Trainium2 Bass kernel for nn_DevLayer_12627203850761 (moe_routing).

Strategy:
  - Batch-parallel across 8 NeuronCores: core c processes batch element c
    of both streams (emb + dis). No collectives needed (routing top-2 and
    per-block weight gather/folding done host-side; `delayed` is a
    per-batch mean so it is core-local).
  - On device, activations live feature-major ([D partitions, T free]) in
    bf16; all matmuls use the weights as the stationary lhsT operand.
  - LayerNorm mean/var are computed with ones-matmuls on the PE (free on
    the bottleneck-adjacent engines), rstd via a bit-hack+Newton rsqrt on
    the vector engine (ACT Rsqrt is banned), broadcast back over
    partitions with a K=1 ones-matmul.
  - LN gamma/beta, biases, torsion factors and the 0.5/0.3 residual
    scales are folded into the weights / per-feature bias vectors on the
    host, so the device only does: stats, center, scale, matmul,
    ACT(tanh/gelu) with per-partition bias, and one fused
    scalar_tensor_tensor per residual add.
  - Layout changes (token-major f32 DRAM <-> feature-major bf16 SBUF) are
    done purely with DMA: SWDGE cast-DMA (f32<->bf16) + HWDGE xbar
    transpose (2-byte dtype).  NOTE: all xbar-transpose DMAs and
    SBUF->SBUF copies must stay on the SAME HWDGE ring (nc.sync) — running
    them concurrently on both rings trips the documented DMA-transpose ||
    SBUF->SBUF hardware hazard and silently corrupts data (observed: rel
    err 0.34 with input transposes moved to the ACT ring).
"""

import sys
import numpy as np

if '/opt/trn_rl_repo' not in sys.path:
    sys.path.insert(0, '/opt/trn_rl_repo')

B, S, D, NB = 8, 8192, 512, 16
P = 128
KB = D // P            # 4 feature blocks
TC = 512               # token chunk (PSUM free dim)
EPS = 1e-5
N_CORES = 8
GELU_FUNC_NAME = "Gelu"   # CoreSim has no Gelu; sim tests swap in "Tanh"

# tuning knobs (consulted at build time; include in cache key)
CFG = {
    "stats_ps_bufs": 3,
    "mm_ps_bufs": 5,
    "rc_bufs_extra": 2,      # rc bufs = GRP + this
    "newton_iters": 1,
    "lookahead_extra": 1,    # L = GRP + this
}

_MODULE_CACHE = {}


# ----------------------------------------------------------------------------
# Host-side routing + weight folding
# ----------------------------------------------------------------------------

def _top2(scores_row):
    # jax.lax.top_k: descending values, ties -> lower index first
    idx = np.lexsort((np.arange(scores_row.shape[0]), -scores_row))
    return int(idx[0]), int(idx[1])


def _prep_host(inputs):
    """Compute routing and folded per-core device inputs."""
    f32 = np.float32
    emb_input = np.asarray(inputs["emb_input"], f32)
    dis_input = np.asarray(inputs["dis_input"], f32)
    torsion = np.asarray(inputs["torsion"], f32)
    dis_on = bool(int(inputs["dis_unlocked"]))

    # ---- routing (sigmoid is monotonic -> top_k on logits)
    m0 = emb_input[0].mean(axis=0, dtype=f32)                       # [D]
    es = m0 @ np.asarray(inputs["emb_sel_W"], f32) + np.asarray(inputs["emb_sel_b"], f32)
    etop = _top2(es)

    # ---- emb folded weights (shared across cores)
    w_e1 = np.empty((2, D, D), f32)
    b_e1 = np.empty((2, D), f32)
    w_e2_base = np.empty((2, D, D), f32)
    b_e2_base = np.empty((2, D), f32)
    for i, idx in enumerate(etop):
        g = np.asarray(inputs["emb_ln_g"], f32)[idx]
        b = np.asarray(inputs["emb_ln_b"], f32)[idx]
        w1 = np.asarray(inputs["emb_w1"], f32)[idx]
        w_e1[i] = g[:, None] * w1
        b_e1[i] = b @ w1 + np.asarray(inputs["emb_b1"], f32)[idx]
        w_e2_base[i] = np.asarray(inputs["emb_w2"], f32)[idx]
        b_e2_base[i] = np.asarray(inputs["emb_b2"], f32)[idx]

    per_core = []
    bf = np.dtype('bfloat16') if hasattr(np, 'bfloat16') else None
    import ml_dtypes
    bf16 = ml_dtypes.bfloat16

    if dis_on:
        dm0 = dis_input[0].mean(axis=0, dtype=f32)
        ds = dm0 @ np.asarray(inputs["dis_sel_W"], f32) + np.asarray(inputs["dis_sel_b"], f32)
        dtop = _top2(ds)
        w_at_base = np.empty((2, D, D), f32)
        ab_base = np.empty((2, D), f32)       # ln1_b @ attnW_g + attn_b
        w_f1 = np.empty((2, D, 2 * D), f32)
        b_f1 = np.empty((2, 2 * D), f32)
        w_f2 = np.empty((2, 2 * D, D), f32)
        b_f2h = np.empty((2, D), f32)
        for i, idx in enumerate(dtop):
            g1 = np.asarray(inputs["dis_ln1_g"], f32)[idx]
            b1 = np.asarray(inputs["dis_ln1_b"], f32)[idx]
            aw = np.asarray(inputs["dis_attn_W"], f32)[idx]
            w_at_base[i] = g1[:, None] * aw
            ab_base[i] = b1 @ aw + np.asarray(inputs["dis_attn_b"], f32)[idx]
            g2 = np.asarray(inputs["dis_ln2_g"], f32)[idx]
            b2 = np.asarray(inputs["dis_ln2_b"], f32)[idx]
            f1 = np.asarray(inputs["dis_ff1_W"], f32)[idx]
            w_f1[i] = g2[:, None] * f1
            b_f1[i] = b2 @ f1 + np.asarray(inputs["dis_ff1_b"], f32)[idx]
            w_f2[i] = 0.5 * np.asarray(inputs["dis_ff2_W"], f32)[idx]
            b_f2h[i] = 0.5 * np.asarray(inputs["dis_ff2_b"], f32)[idx]
        w_f1_bf = w_f1.astype(bf16)
        w_f2_bf = w_f2.astype(bf16)

    w_e1_bf = w_e1.astype(bf16)

    for c in range(N_CORES):
        t_emb3 = 0.3 * (1.0 + 0.1 * torsion[c])      # [D]
        w_e2 = (w_e2_base * t_emb3[None, None, :]).astype(bf16)
        b_e2s = (b_e2_base * t_emb3[None, :]).astype(f32)

        d = {
            "x_emb": np.ascontiguousarray(emb_input[c]),
            "w_e1": w_e1_bf,
            "w_e2": w_e2,
        }
        # vec512 layout: [be1_0, be1_1, be2s_0, be2s_1, ab_0, ab_1, dsc, bf2_0, bf2_1]
        vec512 = np.zeros((9, D), f32)
        vec512[0] = b_e1[0]
        vec512[1] = b_e1[1]
        vec512[2] = b_e2s[0]
        vec512[3] = b_e2s[1]

        if dis_on:
            td05 = 0.5 * (1.0 + 0.05 * torsion[c])   # [D]
            w_at = (w_at_base * td05[None, None, :]).astype(bf16)
            vec512[4] = td05 * ab_base[0]
            vec512[5] = td05 * ab_base[1]
            vec512[6] = td05 * 0.2 / S               # multiplies delayed SUM
            vec512[7] = b_f2h[0]
            vec512[8] = b_f2h[1]
            vec1024 = np.stack([b_f1[0], b_f1[1]]).astype(f32)
            d.update({
                "x_dis": np.ascontiguousarray(dis_input[c]),
                "w_at": w_at,
                "w_f1": w_f1_bf,
                "w_f2": w_f2_bf,
                "vec1024": vec1024,
            })
        d["vec512"] = vec512
        per_core.append(d)
    return per_core, dis_on


# ----------------------------------------------------------------------------
# Device program
# ----------------------------------------------------------------------------

def _build_module(T, dis_on):
    import concourse.bass as bass
    import concourse.mybir as mybir
    import concourse.tile as tile
    from concourse import bacc
    from contextlib import ExitStack

    f32 = mybir.dt.float32
    bf16 = mybir.dt.bfloat16
    i32 = mybir.dt.int32
    Alu = mybir.AluOpType
    Act = mybir.ActivationFunctionType

    NCH = T // TC
    GRP = min(4, NCH)
    NG = NCH // GRP
    # graduated input-group sizes: small first groups so compute starts early
    GS = []
    rem = T
    for sz in (512, 512, 1024):
        if rem > 2048 and sz <= rem:
            GS.append(sz)
            rem -= sz
    while rem > 0:
        sz = min(2048, rem)
        GS.append(sz)
        rem -= sz
    GOFF = [0]
    for sz in GS:
        GOFF.append(GOFF[-1] + sz)

    nc = bacc.Bacc("TRN2", target_bir_lowering=False, debug=False,
                   num_devices=N_CORES)

    x_emb = nc.dram_tensor("x_emb", [T, D], f32, kind="ExternalInput")
    w_e1 = nc.dram_tensor("w_e1", [2, D, D], bf16, kind="ExternalInput")
    w_e2 = nc.dram_tensor("w_e2", [2, D, D], bf16, kind="ExternalInput")
    vec512 = nc.dram_tensor("vec512", [9, D], f32, kind="ExternalInput")
    y_emb = nc.dram_tensor("y_emb", [T, D], f32, kind="ExternalOutput")
    s_tok_e = nc.dram_tensor("s_tok_e", [T, D], bf16, kind="Internal")
    s_feat_e = nc.dram_tensor("s_feat_e", [D, T], bf16, kind="Internal")
    if dis_on:
        x_dis = nc.dram_tensor("x_dis", [T, D], f32, kind="ExternalInput")
        w_at = nc.dram_tensor("w_at", [2, D, D], bf16, kind="ExternalInput")
        w_f1 = nc.dram_tensor("w_f1", [2, D, 2 * D], bf16, kind="ExternalInput")
        w_f2 = nc.dram_tensor("w_f2", [2, 2 * D, D], bf16, kind="ExternalInput")
        vec1024 = nc.dram_tensor("vec1024", [2, 2 * D], f32, kind="ExternalInput")
        y_dis = nc.dram_tensor("y_dis", [T, D], f32, kind="ExternalOutput")
        s_tok_d = nc.dram_tensor("s_tok_d", [T, D], bf16, kind="Internal")
        s_feat_d = nc.dram_tensor("s_feat_d", [D, T], bf16, kind="Internal")

    with tile.TileContext(nc) as tc, ExitStack() as ctx:
        sb = ctx.enter_context(tc.tile_pool(name="sb", bufs=1))
        psum = ctx.enter_context(tc.tile_pool(name="psum", bufs=1, space="PSUM"))

        # ---- constants
        ones_sc = sb.tile([P, P], bf16, tag="ones_sc", name="ones_sc")
        nc.vector.memset(ones_sc, 1.0 / D)
        ones_row = sb.tile([1, P], bf16, tag="ones_row", name="ones_row")
        nc.vector.memset(ones_row, 1.0)
        magic = sb.tile([P, TC], i32, tag="magic", name="magic")
        nc.vector.memset(magic, 0x5f3759df)
        eps_t = sb.tile([P, 1], f32, tag="eps_t", name="eps_t")
        nc.vector.memset(eps_t, EPS)

        # ---- small vectors [128, 9, 4]
        v512 = sb.tile([P, 9, KB], f32, tag="v512", name="v512")
        nc.sync.dma_start(out=v512, in_=vec512[:, :].rearrange("v (a p) -> p v a", p=P))

        def vec_ap(v, mb):
            return v512[:, v, mb:mb + 1]

        if dis_on:
            v1024 = sb.tile([P, 2, 8], f32, tag="v1024", name="v1024")
            nc.sync.dma_start(out=v1024, in_=vec1024[:, :].rearrange("v (a p) -> p v a", p=P))

        # ---- weights (feature-major lhsT layout [P, kb, m])
        def load_w(handle, i, kblocks, mtot, tag, bufs=1):
            t = sb.tile([P, kblocks, mtot], bf16, tag=tag, name=f"{tag}_ld", bufs=bufs)
            nc.sync.dma_start(
                out=t, in_=handle[i:i + 1].rearrange("o (a p) m -> p (o a) m", p=P))
            return t

        we1 = [load_w(w_e1, i, KB, D, f"wA{i}", bufs=1) for i in range(2)]
        we2 = [load_w(w_e2, i, KB, D, f"wA{2 + i}", bufs=1) for i in range(2)]
        if dis_on:
            wf1 = [load_w(w_f1, i, KB, 2 * D, f"wf1_{i}") for i in range(2)]
            wf2 = [load_w(w_f2, i, 2 * KB, D, f"wf2_{i}") for i in range(2)]
            wat = None   # loaded later into the wA slots (after emb finishes)

        # ---- residual stream: per-(pblock, token-group) tiles so slot reuse
        # (emb -> dis) and load/compute overlap happen at group granularity
        NGRP = len(GS)
        import bisect

        def group_of_chunk(k):
            g = bisect.bisect_right(GOFF, k * TC) - 1
            return g, k * TC - GOFF[g]

        LAST_CHUNK_OF_GROUP = {(GOFF[g + 1] // TC) - 1: g for g in range(NGRP)}

        class HStream:
            def __init__(self, which):
                self.which = which
                self.groups = [[None] * NGRP for _ in range(KB)]

            def alloc_group(self, g):
                for pb in range(KB):
                    self.groups[pb][g] = sb.tile(
                        [P, GS[g]], bf16, tag=f"h{pb}g{g}",
                        name=f"h_{self.which}{pb}g{g}")

            def ap(self, pb, k):
                g, off = group_of_chunk(k)
                t = self.groups[pb][g]
                return t[:, off:off + TC]

        def load_group(hs, x_h, s_tok, g):
            sl = slice(GOFF[g], GOFF[g + 1])
            nc.gpsimd.dma_start(out=s_tok[sl, :], in_=x_h[sl, :])  # f32 -> bf16
            for pb in range(KB):
                nc.sync.dma_start(out=hs.groups[pb][g],
                                  in_=s_tok[sl, P * pb:P * (pb + 1)],
                                  transpose=True)

        def store_chunk(hs, s_feat, y_h, k, ck):
            for pb in range(KB):
                nc.sync.dma_start(out=s_feat[P * pb:P * (pb + 1), ck],
                                  in_=hs.ap(pb, k))
            ot = sb.tile([P, KB, D], bf16, tag="ot", bufs=1, name="ot")
            for a in range(KB):
                t0 = k * TC + a * P
                nc.sync.dma_start(out=ot[:, a, :],
                                  in_=s_feat[:, t0:t0 + P], transpose=True)
            nc.gpsimd.dma_start(
                out=y_h[k * TC:(k + 1) * TC, :].rearrange("(a p) d -> p a d", p=P),
                in_=ot)  # bf16 -> f32

        # ---- LN stats machinery
        def newton_rsqrt(st):
            """st: [P, TC] f32 (var+eps, chunk j of the group replicated on
            partitions Wj..W(j+1)) -> [P, TC] bf16 rstd."""
            sh = sb.tile([P, TC], i32, tag="nsh", bufs=1, name="nsh")
            nc.vector.tensor_scalar(out=sh, in0=st.bitcast(i32), scalar1=1,
                                    scalar2=None, op0=Alu.arith_shift_right)
            y = sb.tile([P, TC], f32, tag="ny", bufs=1, name="ny")
            nc.vector.tensor_sub(y.bitcast(i32), magic, sh)
            vh = sb.tile([P, TC], f32, tag="nvh", bufs=1, name="nvh")
            nc.vector.tensor_scalar(out=vh, in0=st, scalar1=-0.5, scalar2=None,
                                    op0=Alu.mult)
            t0 = sb.tile([P, TC], f32, tag="nt0", bufs=1, name="nt0")
            t1 = sb.tile([P, TC], f32, tag="nt1", bufs=1, name="nt1")
            rs = sb.tile([P, TC], bf16, tag="nrs", bufs=2, name="nrs")
            for it in range(CFG["newton_iters"]):
                nc.vector.tensor_mul(t0, y, y)
                nc.vector.tensor_mul(t1, t0, vh)
                nc.vector.tensor_scalar(out=t1, in0=t1, scalar1=1.5, scalar2=None,
                                        op0=Alu.add)
                nc.vector.tensor_mul(rs if it == CFG["newton_iters"] - 1 else y, y, t1)
            return rs

        class LNPhase:
            """One LN + its consumer (matmuls/activations/residual)."""

            def __init__(self, h, main_fn, name, after_chunk=None,
                         sq_dve=False):
                self.h = h          # HStream (stats input / residual)
                self.main_fn = main_fn
                self.name = name
                self.after_chunk = after_chunk
                self.sq_dve = sq_dve
                self.rc = {}
                self.rz = {}

            def stats_chunk(self, k):
                h = self.h
                j = k % GRP
                if j == 0:
                    self._st = sb.tile([P, TC], f32, tag="st", bufs=2, name="st")
                st = self._st
                m_ps = psum.tile([P, TC], f32, tag="stats_ps",
                                 bufs=CFG["stats_ps_bufs"], name="m_ps")
                for kb in range(KB):
                    nc.tensor.matmul(m_ps, ones_sc, h.ap(kb, k),
                                     start=kb == 0, stop=kb == KB - 1)
                m_b = sb.tile([P, TC], bf16, tag="m_b", bufs=2, name="m_b")
                nc.scalar.copy(m_b, m_ps)
                rcs = []
                v_ps = psum.tile([P, TC], f32, tag="stats_ps",
                                 bufs=CFG["stats_ps_bufs"], name="v_ps")
                for kb in range(KB):
                    rc = sb.tile([P, TC], bf16, tag=f"rc{kb}",
                                 bufs=GRP + CFG["rc_bufs_extra"], name=f"rc{kb}")
                    nc.vector.tensor_sub(rc, h.ap(kb, k), m_b)
                    rcs.append(rc)
                    x2 = sb.tile([P, TC], bf16, tag="x2", bufs=2, name="x2")
                    if self.sq_dve:
                        nc.vector.tensor_mul(x2, rc, rc)
                    else:
                        nc.scalar.square(x2, rc)
                    nc.tensor.matmul(v_ps, ones_sc, x2,
                                     start=kb == 0, stop=kb == KB - 1)
                self.rc[k] = rcs
                W = P // GRP
                nc.scalar.activation(st[W * j:W * (j + 1), :], v_ps[0:W, :],
                                     Act.Identity, bias=eps_t[0:W, 0:1],
                                     scale=1.0)
                if j == GRP - 1:
                    rs = newton_rsqrt(st)
                    for jj in range(GRP):
                        kk = k - (GRP - 1) + jj
                        rz = sb.tile([1, TC], bf16, tag="rz", bufs=GRP + 2,
                                     name="rz")
                        nc.sync.dma_start(out=rz, in_=rs[W * jj:W * jj + 1, :])
                        self.rz[kk] = rz

            def main_chunk(self, k):
                ck = slice(k * TC, (k + 1) * TC)
                rb_ps = psum.tile([P, TC], f32, tag="stats_ps",
                                  bufs=CFG["stats_ps_bufs"], name="rb_ps")
                nc.tensor.matmul(rb_ps, ones_row, self.rz.pop(k),
                                 start=True, stop=True)
                rstd_b = sb.tile([P, TC], bf16, tag="rstd_b", bufs=2,
                                 name="rstd_b")
                nc.scalar.copy(rstd_b, rb_ps)
                rcs = self.rc.pop(k)
                xh = []
                for kb in range(KB):
                    t = sb.tile([P, TC], bf16, tag=f"xh{kb}", bufs=2,
                                name=f"xh{kb}")
                    nc.vector.tensor_mul(t, rcs[kb], rstd_b)
                    xh.append(t)
                self.main_fn(k, ck, xh)
                if self.after_chunk is not None:
                    self.after_chunk(k, ck)

        Add = Alu.add

        def emb_main(i):
            def fn(k, ck, xh):
                u_list = []
                for mb in range(KB):
                    u_ps = psum.tile([P, TC], f32, tag="mm_ps",
                                     bufs=CFG["mm_ps_bufs"], name="u_ps")
                    for kb in range(KB):
                        nc.tensor.matmul(u_ps, we1[i][:, kb, P * mb:P * (mb + 1)],
                                         xh[kb], start=kb == 0, stop=kb == KB - 1)
                    u_list.append(u_ps)
                a_list = []
                for mb in range(KB):
                    a = sb.tile([P, TC], bf16, tag=f"a{mb}", bufs=2, name=f"a{mb}")
                    nc.scalar.activation(a, u_list[mb], Act.Tanh,
                                         bias=vec_ap(i, mb), scale=1.0)
                    a_list.append(a)
                for mb in range(KB):
                    v_ps = psum.tile([P, TC], f32, tag="mm_ps",
                                     bufs=CFG["mm_ps_bufs"], name="v_ps2")
                    for kb in range(KB):
                        nc.tensor.matmul(v_ps, we2[i][:, kb, P * mb:P * (mb + 1)],
                                         a_list[kb], start=kb == 0, stop=kb == KB - 1)
                    nc.vector.scalar_tensor_tensor(
                        out=hE.ap(mb, k), in0=v_ps, scalar=vec_ap(2 + i, mb),
                        in1=hE.ap(mb, k), op0=Add, op1=Add)
            return fn

        def dis_attn_main(i):
            def fn(k, ck, xh):
                for mb in range(KB):
                    u_ps = psum.tile([P, TC], f32, tag="mm_ps",
                                     bufs=CFG["mm_ps_bufs"], name="ua_ps")
                    for kb in range(KB):
                        nc.tensor.matmul(u_ps, wat[i][:, kb, P * mb:P * (mb + 1)],
                                         xh[kb], start=kb == 0, stop=kb == KB - 1)
                    nc.vector.scalar_tensor_tensor(
                        out=hD.ap(mb, k), in0=u_ps, scalar=bias_dis[i][:, mb:mb + 1],
                        in1=hD.ap(mb, k), op0=Add, op1=Add)
            return fn

        def dis_ff_main(i):
            def fn(k, ck, xh):
                g_list = []
                for mb8 in range(2 * KB):
                    g_ps = psum.tile([P, TC], f32, tag="mm_ps",
                                     bufs=CFG["mm_ps_bufs"], name="g_ps")
                    for kb in range(KB):
                        nc.tensor.matmul(g_ps, wf1[i][:, kb, P * mb8:P * (mb8 + 1)],
                                         xh[kb], start=kb == 0, stop=kb == KB - 1)
                    gt = sb.tile([P, TC], bf16, tag=f"g{mb8}", bufs=2, name=f"g{mb8}")
                    nc.scalar.activation(gt, g_ps, getattr(Act, GELU_FUNC_NAME),
                                         bias=v1024[:, i, mb8:mb8 + 1], scale=1.0)
                    g_list.append(gt)
                for mb in range(KB):
                    h2_ps = psum.tile([P, TC], f32, tag="mm_ps",
                                      bufs=CFG["mm_ps_bufs"], name="h2_ps")
                    for kb8 in range(2 * KB):
                        nc.tensor.matmul(h2_ps, wf2[i][:, kb8, P * mb:P * (mb + 1)],
                                         g_list[kb8], start=kb8 == 0,
                                         stop=kb8 == 2 * KB - 1)
                    nc.vector.scalar_tensor_tensor(
                        out=hD.ap(mb, k), in0=h2_ps, scalar=vec_ap(7 + i, mb),
                        in1=hD.ap(mb, k), op0=Add, op1=Add)
            return fn

        # ---- streams + hooks
        hE = HStream("e")
        for g in range(NGRP):
            hE.alloc_group(g)
            load_group(hE, x_emb, s_tok_e, g)

        if dis_on:
            hD = HStream("d")
            bias_dis = []
            dsum = [sb.tile([P, NGRP], f32, tag=f"dsum{pb}", name=f"dsum{pb}")
                    for pb in range(KB)]

            def dis_prep_hook(k, ck):
                nonlocal wat
                # after emb's final phase finishes group g, reuse the slots
                # for the dis stream and do the per-group delayed-sum
                if k not in LAST_CHUNK_OF_GROUP:
                    return
                g = LAST_CHUNK_OF_GROUP[k]
                if g == 0:
                    wat = [load_w(w_at, i, KB, D, f"wA{i}") for i in range(2)]
                hD.alloc_group(g)
                load_group(hD, x_dis, s_tok_d, g)
                for pb in range(KB):
                    nc.vector.tensor_reduce(out=dsum[pb][:, g:g + 1],
                                            in_=hD.groups[pb][g],
                                            axis=mybir.AxisListType.X, op=Alu.add)
                if g == NGRP - 1:
                    dsfin = [sb.tile([P, 1], f32, tag=f"dsf{pb}", name=f"dsf{pb}")
                             for pb in range(KB)]
                    for pb in range(KB):
                        nc.vector.tensor_reduce(out=dsfin[pb], in_=dsum[pb],
                                                axis=mybir.AxisListType.X,
                                                op=Alu.add)
                    for i in range(2):
                        bd = sb.tile([P, KB], f32, tag=f"bias_dis{i}",
                                     name=f"bias_dis{i}")
                        for mb in range(KB):
                            nc.vector.tensor_scalar(
                                out=bd[:, mb:mb + 1], in0=dsfin[mb],
                                scalar1=vec_ap(6, mb), scalar2=vec_ap(4 + i, mb),
                                op0=Alu.mult, op1=Alu.add)
                        bias_dis.append(bd)

            def e1_hook(k, ck):
                store_chunk(hE, s_feat_e, y_emb, k, ck)
                dis_prep_hook(k, ck)
        else:
            def e1_hook(k, ck):
                store_chunk(hE, s_feat_e, y_emb, k, ck)

        phases = [LNPhase(hE, emb_main(0), "e0"),
                  LNPhase(hE, emb_main(1), "e1", after_chunk=e1_hook)]
        if dis_on:
            phases += [
                LNPhase(hD, dis_attn_main(0), "d0a"),
                LNPhase(hD, dis_ff_main(0), "d0f", sq_dve=True),
                LNPhase(hD, dis_attn_main(1), "d1a"),
                LNPhase(hD, dis_ff_main(1), "d1f", sq_dve=True,
                        after_chunk=lambda k, ck: store_chunk(hD, s_feat_d, y_dis, k, ck)),
            ]

        def emit(phs):
            # software-pipelined emission at chunk granularity: stats run
            # L chunks ahead of main.  Requires NCH > L so cross-phase stats
            # never precede the main that produces their input; otherwise
            # fall back to serial per-phase emission.
            sq = [(ph, k) for ph in phs for k in range(NCH)]
            L = GRP + CFG["lookahead_extra"]
            if NCH <= L:
                for ph in phs:
                    for k in range(NCH):
                        ph.stats_chunk(k)
                    for k in range(NCH):
                        ph.main_chunk(k)
                return
            for i, (ph, k) in enumerate(sq):
                ph.stats_chunk(k)
                if i - L >= 0:
                    pj, kj = sq[i - L]
                    pj.main_chunk(kj)
            for i in range(len(sq) - L, len(sq)):
                pj, kj = sq[i]
                pj.main_chunk(kj)

        emit(phases)

    nc.compile()
    return nc


# ----------------------------------------------------------------------------
# Entry point
# ----------------------------------------------------------------------------

def _get_module(T, dis_on):
    key = (T, dis_on, GELU_FUNC_NAME)
    if key not in _MODULE_CACHE:
        _MODULE_CACHE[key] = _build_module(T, dis_on)
    return _MODULE_CACHE[key]


LAST_EXEC_TIME_NS = None
TRACE = False


def kernel(**inputs):
    global LAST_EXEC_TIME_NS
    from concourse.bass_utils import run_bass_kernel_spmd

    per_core, dis_on = _prep_host(inputs)
    nc = _get_module(S, dis_on)

    res = run_bass_kernel_spmd(nc, per_core, core_ids=list(range(N_CORES)),
                               trace=TRACE)
    LAST_EXEC_TIME_NS = res.exec_time_ns

    emb = np.stack([res.results[c]["y_emb"] for c in range(N_CORES)])
    if dis_on:
        dis = np.stack([res.results[c]["y_dis"] for c in range(N_CORES)])
    else:
        dis = None
    return emb, dis


# revision 34
# speedup vs baseline: 113.6578x; 1.0029x over previous
"""Trainium2 Bass kernel for nn_DevLayer_12627203850761 (moe_routing).

Strategy:
  - Batch-parallel across 8 NeuronCores: core c processes batch element c
    of both streams (emb + dis). No collectives needed (routing top-2 and
    per-block weight gather/folding done host-side; `delayed` is a
    per-batch mean so it is core-local).
  - On device, activations live feature-major ([D partitions, T free]) in
    bf16; all matmuls use the weights as the stationary lhsT operand.
  - LayerNorm mean/var are computed with ones-matmuls on the PE (free on
    the bottleneck-adjacent engines), rstd via a bit-hack+Newton rsqrt on
    the vector engine (ACT Rsqrt is banned), broadcast back over
    partitions with a K=1 ones-matmul.
  - LN gamma/beta, biases, torsion factors and the 0.5/0.3 residual
    scales are folded into the weights / per-feature bias vectors on the
    host, so the device only does: stats, center, scale, matmul,
    ACT(tanh/gelu) with per-partition bias, and one fused
    scalar_tensor_tensor per residual add.
  - Layout changes (token-major f32 DRAM <-> feature-major bf16 SBUF) are
    done purely with DMA: SWDGE cast-DMA (f32<->bf16) + HWDGE xbar
    transpose (2-byte dtype).  NOTE: all xbar-transpose DMAs and
    SBUF->SBUF copies must stay on the SAME HWDGE ring (nc.sync) — running
    them concurrently on both rings trips the documented DMA-transpose ||
    SBUF->SBUF hardware hazard and silently corrupts data (observed: rel
    err 0.34 with input transposes moved to the ACT ring).
"""

import sys
import numpy as np

if '/opt/trn_rl_repo' not in sys.path:
    sys.path.insert(0, '/opt/trn_rl_repo')

B, S, D, NB = 8, 8192, 512, 16
P = 128
KB = D // P            # 4 feature blocks
TC = 512               # token chunk (PSUM free dim)
EPS = 1e-5
N_CORES = 8
GELU_FUNC_NAME = "Gelu"   # CoreSim has no Gelu; sim tests swap in "Tanh"

# tuning knobs (consulted at build time; include in cache key)
CFG = {
    "stats_ps_bufs": 3,
    "mm_ps_bufs": 5,
    "rc_bufs_extra": 2,      # rc bufs = GRP + this
    "newton_iters": 1,
    "lookahead_extra": 1,    # L = GRP + this
}

_MODULE_CACHE = {}


# ----------------------------------------------------------------------------
# Host-side routing + weight folding
# ----------------------------------------------------------------------------

def _top2(scores_row):
    # jax.lax.top_k: descending values, ties -> lower index first
    idx = np.lexsort((np.arange(scores_row.shape[0]), -scores_row))
    return int(idx[0]), int(idx[1])


def _prep_host(inputs):
    """Compute routing and folded per-core device inputs."""
    f32 = np.float32
    emb_input = np.asarray(inputs["emb_input"], f32)
    dis_input = np.asarray(inputs["dis_input"], f32)
    torsion = np.asarray(inputs["torsion"], f32)
    dis_on = bool(int(inputs["dis_unlocked"]))

    # ---- routing (sigmoid is monotonic -> top_k on logits)
    m0 = emb_input[0].mean(axis=0, dtype=f32)                       # [D]
    es = m0 @ np.asarray(inputs["emb_sel_W"], f32) + np.asarray(inputs["emb_sel_b"], f32)
    etop = _top2(es)

    # ---- emb folded weights (shared across cores)
    w_e1 = np.empty((2, D, D), f32)
    b_e1 = np.empty((2, D), f32)
    w_e2_base = np.empty((2, D, D), f32)
    b_e2_base = np.empty((2, D), f32)
    for i, idx in enumerate(etop):
        g = np.asarray(inputs["emb_ln_g"], f32)[idx]
        b = np.asarray(inputs["emb_ln_b"], f32)[idx]
        w1 = np.asarray(inputs["emb_w1"], f32)[idx]
        w_e1[i] = g[:, None] * w1
        b_e1[i] = b @ w1 + np.asarray(inputs["emb_b1"], f32)[idx]
        w_e2_base[i] = np.asarray(inputs["emb_w2"], f32)[idx]
        b_e2_base[i] = np.asarray(inputs["emb_b2"], f32)[idx]

    per_core = []
    bf = np.dtype('bfloat16') if hasattr(np, 'bfloat16') else None
    import ml_dtypes
    bf16 = ml_dtypes.bfloat16

    if dis_on:
        dm0 = dis_input[0].mean(axis=0, dtype=f32)
        ds = dm0 @ np.asarray(inputs["dis_sel_W"], f32) + np.asarray(inputs["dis_sel_b"], f32)
        dtop = _top2(ds)
        w_at_base = np.empty((2, D, D), f32)
        ab_base = np.empty((2, D), f32)       # ln1_b @ attnW_g + attn_b
        w_f1 = np.empty((2, D, 2 * D), f32)
        b_f1 = np.empty((2, 2 * D), f32)
        w_f2 = np.empty((2, 2 * D, D), f32)
        b_f2h = np.empty((2, D), f32)
        for i, idx in enumerate(dtop):
            g1 = np.asarray(inputs["dis_ln1_g"], f32)[idx]
            b1 = np.asarray(inputs["dis_ln1_b"], f32)[idx]
            aw = np.asarray(inputs["dis_attn_W"], f32)[idx]
            w_at_base[i] = g1[:, None] * aw
            ab_base[i] = b1 @ aw + np.asarray(inputs["dis_attn_b"], f32)[idx]
            g2 = np.asarray(inputs["dis_ln2_g"], f32)[idx]
            b2 = np.asarray(inputs["dis_ln2_b"], f32)[idx]
            f1 = np.asarray(inputs["dis_ff1_W"], f32)[idx]
            w_f1[i] = g2[:, None] * f1
            b_f1[i] = b2 @ f1 + np.asarray(inputs["dis_ff1_b"], f32)[idx]
            w_f2[i] = 0.5 * np.asarray(inputs["dis_ff2_W"], f32)[idx]
            b_f2h[i] = 0.5 * np.asarray(inputs["dis_ff2_b"], f32)[idx]
        w_f1_bf = w_f1.astype(bf16)
        w_f2_bf = w_f2.astype(bf16)

    w_e1_bf = w_e1.astype(bf16)

    for c in range(N_CORES):
        t_emb3 = 0.3 * (1.0 + 0.1 * torsion[c])      # [D]
        w_e2 = (w_e2_base * t_emb3[None, None, :]).astype(bf16)
        b_e2s = (b_e2_base * t_emb3[None, :]).astype(f32)

        d = {
            "x_emb": np.ascontiguousarray(emb_input[c]),
            "w_e1": w_e1_bf,
            "w_e2": w_e2,
        }
        # vec512 layout: [be1_0, be1_1, be2s_0, be2s_1, ab_0, ab_1, dsc, bf2_0, bf2_1]
        vec512 = np.zeros((9, D), f32)
        vec512[0] = b_e1[0]
        vec512[1] = b_e1[1]
        vec512[2] = b_e2s[0]
        vec512[3] = b_e2s[1]

        if dis_on:
            td05 = 0.5 * (1.0 + 0.05 * torsion[c])   # [D]
            w_at = (w_at_base * td05[None, None, :]).astype(bf16)
            vec512[4] = td05 * ab_base[0]
            vec512[5] = td05 * ab_base[1]
            vec512[6] = td05 * 0.2 / S               # multiplies delayed SUM
            vec512[7] = b_f2h[0]
            vec512[8] = b_f2h[1]
            vec1024 = np.stack([b_f1[0], b_f1[1]]).astype(f32)
            d.update({
                "x_dis": np.ascontiguousarray(dis_input[c]),
                "w_at": w_at,
                "w_f1": w_f1_bf,
                "w_f2": w_f2_bf,
                "vec1024": vec1024,
            })
        d["vec512"] = vec512
        per_core.append(d)
    return per_core, dis_on


# ----------------------------------------------------------------------------
# Device program
# ----------------------------------------------------------------------------

def _build_module(T, dis_on):
    import concourse.bass as bass
    import concourse.mybir as mybir
    import concourse.tile as tile
    from concourse import bacc
    from contextlib import ExitStack

    f32 = mybir.dt.float32
    bf16 = mybir.dt.bfloat16
    i32 = mybir.dt.int32
    Alu = mybir.AluOpType
    Act = mybir.ActivationFunctionType

    NCH = T // TC
    GRP = min(4, NCH)
    NG = NCH // GRP
    # graduated input-group sizes: small first groups so compute starts early
    GS = []
    rem = T
    for sz in (512, 512, 1024):
        if rem > 2048 and sz <= rem:
            GS.append(sz)
            rem -= sz
    while rem > 0:
        sz = min(2048, rem)
        GS.append(sz)
        rem -= sz
    GOFF = [0]
    for sz in GS:
        GOFF.append(GOFF[-1] + sz)

    nc = bacc.Bacc("TRN2", target_bir_lowering=False, debug=False,
                   num_devices=N_CORES)

    x_emb = nc.dram_tensor("x_emb", [T, D], f32, kind="ExternalInput")
    w_e1 = nc.dram_tensor("w_e1", [2, D, D], bf16, kind="ExternalInput")
    w_e2 = nc.dram_tensor("w_e2", [2, D, D], bf16, kind="ExternalInput")
    vec512 = nc.dram_tensor("vec512", [9, D], f32, kind="ExternalInput")
    y_emb = nc.dram_tensor("y_emb", [T, D], f32, kind="ExternalOutput")
    s_tok_e = nc.dram_tensor("s_tok_e", [T, D], bf16, kind="Internal")
    s_feat_e = nc.dram_tensor("s_feat_e", [D, T], bf16, kind="Internal")
    if dis_on:
        x_dis = nc.dram_tensor("x_dis", [T, D], f32, kind="ExternalInput")
        w_at = nc.dram_tensor("w_at", [2, D, D], bf16, kind="ExternalInput")
        w_f1 = nc.dram_tensor("w_f1", [2, D, 2 * D], bf16, kind="ExternalInput")
        w_f2 = nc.dram_tensor("w_f2", [2, 2 * D, D], bf16, kind="ExternalInput")
        vec1024 = nc.dram_tensor("vec1024", [2, 2 * D], f32, kind="ExternalInput")
        y_dis = nc.dram_tensor("y_dis", [T, D], f32, kind="ExternalOutput")
        s_tok_d = nc.dram_tensor("s_tok_d", [T, D], bf16, kind="Internal")
        s_feat_d = nc.dram_tensor("s_feat_d", [D, T], bf16, kind="Internal")

    with tile.TileContext(nc) as tc, ExitStack() as ctx:
        sb = ctx.enter_context(tc.tile_pool(name="sb", bufs=1))
        psum = ctx.enter_context(tc.tile_pool(name="psum", bufs=1, space="PSUM"))

        # ---- constants
        ones_sc = sb.tile([P, P], bf16, tag="ones_sc", name="ones_sc")
        nc.vector.memset(ones_sc, 1.0 / D)
        ones_row = sb.tile([1, P], bf16, tag="ones_row", name="ones_row")
        nc.vector.memset(ones_row, 1.0)
        magic = sb.tile([P, TC], i32, tag="magic", name="magic")
        nc.vector.memset(magic, 0x5f3759df)
        eps_t = sb.tile([P, 1], f32, tag="eps_t", name="eps_t")
        nc.vector.memset(eps_t, EPS)

        # ---- small vectors [128, 9, 4]
        v512 = sb.tile([P, 9, KB], f32, tag="v512", name="v512")
        nc.sync.dma_start(out=v512, in_=vec512[:, :].rearrange("v (a p) -> p v a", p=P))

        def vec_ap(v, mb):
            return v512[:, v, mb:mb + 1]

        if dis_on:
            v1024 = sb.tile([P, 2, 8], f32, tag="v1024", name="v1024")
            nc.sync.dma_start(out=v1024, in_=vec1024[:, :].rearrange("v (a p) -> p v a", p=P))

        # ---- weights (feature-major lhsT layout [P, kb, m])
        def load_w(handle, i, kblocks, mtot, tag, bufs=1):
            t = sb.tile([P, kblocks, mtot], bf16, tag=tag, name=f"{tag}_ld", bufs=bufs)
            nc.sync.dma_start(
                out=t, in_=handle[i:i + 1].rearrange("o (a p) m -> p (o a) m", p=P))
            return t

        we1 = [load_w(w_e1, i, KB, D, f"wA{i}", bufs=1) for i in range(2)]
        we2 = [load_w(w_e2, i, KB, D, f"wA{2 + i}", bufs=1) for i in range(2)]
        if dis_on:
            wf1 = [load_w(w_f1, i, KB, 2 * D, f"wf1_{i}") for i in range(2)]
            wf2 = [load_w(w_f2, i, 2 * KB, D, f"wf2_{i}") for i in range(2)]
            wat = None   # loaded later into the wA slots (after emb finishes)

        # ---- residual stream: per-(pblock, token-group) tiles so slot reuse
        # (emb -> dis) and load/compute overlap happen at group granularity
        NGRP = len(GS)
        import bisect

        def group_of_chunk(k):
            g = bisect.bisect_right(GOFF, k * TC) - 1
            return g, k * TC - GOFF[g]

        LAST_CHUNK_OF_GROUP = {(GOFF[g + 1] // TC) - 1: g for g in range(NGRP)}

        class HStream:
            def __init__(self, which):
                self.which = which
                self.groups = [[None] * NGRP for _ in range(KB)]

            def alloc_group(self, g):
                for pb in range(KB):
                    self.groups[pb][g] = sb.tile(
                        [P, GS[g]], bf16, tag=f"h{pb}g{g}",
                        name=f"h_{self.which}{pb}g{g}")

            def ap(self, pb, k):
                g, off = group_of_chunk(k)
                t = self.groups[pb][g]
                return t[:, off:off + TC]

        def load_group(hs, x_h, s_tok, g):
            sl = slice(GOFF[g], GOFF[g + 1])
            nc.gpsimd.dma_start(out=s_tok[sl, :], in_=x_h[sl, :])  # f32 -> bf16
            for pb in range(KB):
                nc.sync.dma_start(out=hs.groups[pb][g],
                                  in_=s_tok[sl, P * pb:P * (pb + 1)],
                                  transpose=True)

        def store_chunk(hs, s_feat, y_h, k, ck):
            for pb in range(KB):
                nc.sync.dma_start(out=s_feat[P * pb:P * (pb + 1), ck],
                                  in_=hs.ap(pb, k))
            ot = sb.tile([P, KB, D], bf16, tag="ot", bufs=1, name="ot")
            for a in range(KB):
                t0 = k * TC + a * P
                nc.sync.dma_start(out=ot[:, a, :],
                                  in_=s_feat[:, t0:t0 + P], transpose=True)
            nc.gpsimd.dma_start(
                out=y_h[k * TC:(k + 1) * TC, :].rearrange("(a p) d -> p a d", p=P),
                in_=ot)  # bf16 -> f32

        # ---- LN stats machinery
        def newton_rsqrt(st):
            """st: [P, TC] f32 (var+eps, chunk j of the group replicated on
            partitions Wj..W(j+1)) -> [P, TC] bf16 rstd."""
            sh = sb.tile([P, TC], i32, tag="nsh", bufs=1, name="nsh")
            nc.vector.tensor_scalar(out=sh, in0=st.bitcast(i32), scalar1=1,
                                    scalar2=None, op0=Alu.arith_shift_right)
            y = sb.tile([P, TC], f32, tag="ny", bufs=1, name="ny")
            nc.vector.tensor_sub(y.bitcast(i32), magic, sh)
            vh = sb.tile([P, TC], f32, tag="nvh", bufs=1, name="nvh")
            nc.vector.tensor_scalar(out=vh, in0=st, scalar1=-0.5, scalar2=None,
                                    op0=Alu.mult)
            t0 = sb.tile([P, TC], f32, tag="nt0", bufs=1, name="nt0")
            t1 = sb.tile([P, TC], f32, tag="nt1", bufs=1, name="nt1")
            rs = sb.tile([P, TC], bf16, tag="nrs", bufs=2, name="nrs")
            for it in range(CFG["newton_iters"]):
                nc.vector.tensor_mul(t0, y, y)
                nc.vector.tensor_mul(t1, t0, vh)
                nc.vector.tensor_scalar(out=t1, in0=t1, scalar1=1.5, scalar2=None,
                                        op0=Alu.add)
                nc.vector.tensor_mul(rs if it == CFG["newton_iters"] - 1 else y, y, t1)
            return rs

        class LNPhase:
            """One LN + its consumer (matmuls/activations/residual)."""

            def __init__(self, h, main_fn, name, after_chunk=None,
                         sq_dve=False):
                self.h = h          # HStream (stats input / residual)
                self.main_fn = main_fn
                self.name = name
                self.after_chunk = after_chunk
                self.sq_dve = sq_dve
                self.rc = {}
                self.rz = {}

            def stats_chunk(self, k):
                h = self.h
                j = k % GRP
                if j == 0:
                    self._st = sb.tile([P, TC], f32, tag="st", bufs=2, name="st")
                st = self._st
                m_ps = psum.tile([P, TC], f32, tag="stats_ps",
                                 bufs=CFG["stats_ps_bufs"], name="m_ps")
                for kb in range(KB):
                    nc.tensor.matmul(m_ps, ones_sc, h.ap(kb, k),
                                     start=kb == 0, stop=kb == KB - 1)
                m_b = sb.tile([P, TC], bf16, tag="m_b", bufs=2, name="m_b")
                nc.scalar.copy(m_b, m_ps)
                rcs = []
                v_ps = psum.tile([P, TC], f32, tag="stats_ps",
                                 bufs=CFG["stats_ps_bufs"], name="v_ps")
                for kb in range(KB):
                    rc = sb.tile([P, TC], bf16, tag=f"rc{kb}",
                                 bufs=GRP + CFG["rc_bufs_extra"], name=f"rc{kb}")
                    nc.vector.tensor_sub(rc, h.ap(kb, k), m_b)
                    rcs.append(rc)
                    x2 = sb.tile([P, TC], bf16, tag="x2", bufs=2, name="x2")
                    if self.sq_dve:
                        nc.vector.tensor_mul(x2, rc, rc)
                    else:
                        nc.scalar.square(x2, rc)
                    nc.tensor.matmul(v_ps, ones_sc, x2,
                                     start=kb == 0, stop=kb == KB - 1)
                self.rc[k] = rcs
                W = P // GRP
                nc.scalar.activation(st[W * j:W * (j + 1), :], v_ps[0:W, :],
                                     Act.Identity, bias=eps_t[0:W, 0:1],
                                     scale=1.0)
                if j == GRP - 1:
                    rs = newton_rsqrt(st)
                    for jj in range(GRP):
                        kk = k - (GRP - 1) + jj
                        if jj == 0:
                            # matmul operands must share base partition; the
                            # ones_row lhsT sits at base 0, so only row 0 can
                            # be read directly -- exactly the chunk whose
                            # broadcast gates the group boundary.
                            self.rz[kk] = rs[0:1, :]
                        else:
                            rz = sb.tile([1, TC], bf16, tag="rz", bufs=GRP,
                                         name="rz")
                            nc.sync.dma_start(out=rz, in_=rs[W * jj:W * jj + 1, :])
                            self.rz[kk] = rz

            def main_chunk(self, k):
                ck = slice(k * TC, (k + 1) * TC)
                rb_ps = psum.tile([P, TC], f32, tag="stats_ps",
                                  bufs=CFG["stats_ps_bufs"], name="rb_ps")
                nc.tensor.matmul(rb_ps, ones_row, self.rz.pop(k),
                                 start=True, stop=True)
                rstd_b = sb.tile([P, TC], bf16, tag="rstd_b", bufs=2,
                                 name="rstd_b")
                nc.scalar.copy(rstd_b, rb_ps)
                rcs = self.rc.pop(k)
                xh = []
                for kb in range(KB):
                    t = sb.tile([P, TC], bf16, tag=f"xh{kb}", bufs=2,
                                name=f"xh{kb}")
                    nc.vector.tensor_mul(t, rcs[kb], rstd_b)
                    xh.append(t)
                self.main_fn(k, ck, xh)
                if self.after_chunk is not None:
                    self.after_chunk(k, ck)

        Add = Alu.add

        def emb_main(i):
            def fn(k, ck, xh):
                u_list = []
                for mb in range(KB):
                    u_ps = psum.tile([P, TC], f32, tag="mm_ps",
                                     bufs=CFG["mm_ps_bufs"], name="u_ps")
                    for kb in range(KB):
                        nc.tensor.matmul(u_ps, we1[i][:, kb, P * mb:P * (mb + 1)],
                                         xh[kb], start=kb == 0, stop=kb == KB - 1)
                    u_list.append(u_ps)
                a_list = []
                for mb in range(KB):
                    a = sb.tile([P, TC], bf16, tag=f"a{mb}", bufs=2, name=f"a{mb}")
                    nc.scalar.activation(a, u_list[mb], Act.Tanh,
                                         bias=vec_ap(i, mb), scale=1.0)
                    a_list.append(a)
                for mb in range(KB):
                    v_ps = psum.tile([P, TC], f32, tag="mm_ps",
                                     bufs=CFG["mm_ps_bufs"], name="v_ps2")
                    for kb in range(KB):
                        nc.tensor.matmul(v_ps, we2[i][:, kb, P * mb:P * (mb + 1)],
                                         a_list[kb], start=kb == 0, stop=kb == KB - 1)
                    nc.vector.scalar_tensor_tensor(
                        out=hE.ap(mb, k), in0=v_ps, scalar=vec_ap(2 + i, mb),
                        in1=hE.ap(mb, k), op0=Add, op1=Add)
            return fn

        def dis_attn_main(i):
            def fn(k, ck, xh):
                for mb in range(KB):
                    u_ps = psum.tile([P, TC], f32, tag="mm_ps",
                                     bufs=CFG["mm_ps_bufs"], name="ua_ps")
                    for kb in range(KB):
                        nc.tensor.matmul(u_ps, wat[i][:, kb, P * mb:P * (mb + 1)],
                                         xh[kb], start=kb == 0, stop=kb == KB - 1)
                    nc.vector.scalar_tensor_tensor(
                        out=hD.ap(mb, k), in0=u_ps, scalar=bias_dis[i][:, mb:mb + 1],
                        in1=hD.ap(mb, k), op0=Add, op1=Add)
            return fn

        def dis_ff_main(i):
            def fn(k, ck, xh):
                g_list = []
                for mb8 in range(2 * KB):
                    g_ps = psum.tile([P, TC], f32, tag="mm_ps",
                                     bufs=CFG["mm_ps_bufs"], name="g_ps")
                    for kb in range(KB):
                        nc.tensor.matmul(g_ps, wf1[i][:, kb, P * mb8:P * (mb8 + 1)],
                                         xh[kb], start=kb == 0, stop=kb == KB - 1)
                    gt = sb.tile([P, TC], bf16, tag=f"g{mb8}", bufs=2, name=f"g{mb8}")
                    nc.scalar.activation(gt, g_ps, getattr(Act, GELU_FUNC_NAME),
                                         bias=v1024[:, i, mb8:mb8 + 1], scale=1.0)
                    g_list.append(gt)
                for mb in range(KB):
                    h2_ps = psum.tile([P, TC], f32, tag="mm_ps",
                                      bufs=CFG["mm_ps_bufs"], name="h2_ps")
                    for kb8 in range(2 * KB):
                        nc.tensor.matmul(h2_ps, wf2[i][:, kb8, P * mb:P * (mb + 1)],
                                         g_list[kb8], start=kb8 == 0,
                                         stop=kb8 == 2 * KB - 1)
                    nc.vector.scalar_tensor_tensor(
                        out=hD.ap(mb, k), in0=h2_ps, scalar=vec_ap(7 + i, mb),
                        in1=hD.ap(mb, k), op0=Add, op1=Add)
            return fn

        # ---- streams + hooks
        hE = HStream("e")
        for g in range(NGRP):
            hE.alloc_group(g)
            load_group(hE, x_emb, s_tok_e, g)

        if dis_on:
            hD = HStream("d")
            bias_dis = []
            dsum = [sb.tile([P, NGRP], f32, tag=f"dsum{pb}", name=f"dsum{pb}")
                    for pb in range(KB)]

            def dis_prep_hook(k, ck):
                nonlocal wat
                # after emb's final phase finishes group g, reuse the slots
                # for the dis stream and do the per-group delayed-sum
                if k not in LAST_CHUNK_OF_GROUP:
                    return
                g = LAST_CHUNK_OF_GROUP[k]
                if g == 0:
                    wat = [load_w(w_at, i, KB, D, f"wA{i}") for i in range(2)]
                hD.alloc_group(g)
                load_group(hD, x_dis, s_tok_d, g)
                for pb in range(KB):
                    nc.vector.tensor_reduce(out=dsum[pb][:, g:g + 1],
                                            in_=hD.groups[pb][g],
                                            axis=mybir.AxisListType.X, op=Alu.add)
                if g == NGRP - 1:
                    dsfin = [sb.tile([P, 1], f32, tag=f"dsf{pb}", name=f"dsf{pb}")
                             for pb in range(KB)]
                    for pb in range(KB):
                        nc.vector.tensor_reduce(out=dsfin[pb], in_=dsum[pb],
                                                axis=mybir.AxisListType.X,
                                                op=Alu.add)
                    for i in range(2):
                        bd = sb.tile([P, KB], f32, tag=f"bias_dis{i}",
                                     name=f"bias_dis{i}")
                        for mb in range(KB):
                            nc.vector.tensor_scalar(
                                out=bd[:, mb:mb + 1], in0=dsfin[mb],
                                scalar1=vec_ap(6, mb), scalar2=vec_ap(4 + i, mb),
                                op0=Alu.mult, op1=Alu.add)
                        bias_dis.append(bd)

            def e1_hook(k, ck):
                store_chunk(hE, s_feat_e, y_emb, k, ck)
                dis_prep_hook(k, ck)
        else:
            def e1_hook(k, ck):
                store_chunk(hE, s_feat_e, y_emb, k, ck)

        phases = [LNPhase(hE, emb_main(0), "e0"),
                  LNPhase(hE, emb_main(1), "e1", after_chunk=e1_hook)]
        if dis_on:
            phases += [
                LNPhase(hD, dis_attn_main(0), "d0a"),
                LNPhase(hD, dis_ff_main(0), "d0f", sq_dve=True),
                LNPhase(hD, dis_attn_main(1), "d1a"),
                LNPhase(hD, dis_ff_main(1), "d1f", sq_dve=True,
                        after_chunk=lambda k, ck: store_chunk(hD, s_feat_d, y_dis, k, ck)),
            ]

        def emit(phs):
            # software-pipelined emission at chunk granularity: stats run
            # L chunks ahead of main.  Requires NCH > L so cross-phase stats
            # never precede the main that produces their input; otherwise
            # fall back to serial per-phase emission.
            sq = [(ph, k) for ph in phs for k in range(NCH)]
            L = GRP + CFG["lookahead_extra"]
            if NCH <= L:
                for ph in phs:
                    for k in range(NCH):
                        ph.stats_chunk(k)
                    for k in range(NCH):
                        ph.main_chunk(k)
                return
            for i, (ph, k) in enumerate(sq):
                ph.stats_chunk(k)
                if i - L >= 0:
                    pj, kj = sq[i - L]
                    pj.main_chunk(kj)
            for i in range(len(sq) - L, len(sq)):
                pj, kj = sq[i]
                pj.main_chunk(kj)

        emit(phases)

    nc.compile()
    return nc


# ----------------------------------------------------------------------------
# Entry point
# ----------------------------------------------------------------------------

def _get_module(T, dis_on):
    key = (T, dis_on, GELU_FUNC_NAME)
    if key not in _MODULE_CACHE:
        _MODULE_CACHE[key] = _build_module(T, dis_on)
    return _MODULE_CACHE[key]


LAST_EXEC_TIME_NS = None
TRACE = False


def kernel(**inputs):
    global LAST_EXEC_TIME_NS
    from concourse.bass_utils import run_bass_kernel_spmd

    per_core, dis_on = _prep_host(inputs)
    nc = _get_module(S, dis_on)

    res = run_bass_kernel_spmd(nc, per_core, core_ids=list(range(N_CORES)),
                               trace=TRACE)
    LAST_EXEC_TIME_NS = res.exec_time_ns

    emb = np.stack([res.results[c]["y_emb"] for c in range(N_CORES)])
    if dis_on:
        dis = np.stack([res.results[c]["y_dis"] for c in range(N_CORES)])
    else:
        dis = None
    return emb, dis
